# revision 2
# baseline (speedup 1.0000x reference)
"""Multi-head causal attention (B=2, T=4096, D=1024, H=16) on 8 trn2 cores.

Sharding: core c = 4*b + g handles batch b and head-group g (4 heads).
Merged single-pass pipeline per 512-col q-block tb:
  proj(tb) -> norm(tb-1) -> oproj(tb-1) -> SDPA(i=tb)
SDPA streams in bf16 (q/k/v/probs); Z rows fused into PV via [v|ones]
stationary tiles; causal mask via DVE multiply with a triangular bf16
constant. Host sums the per-core partial O^T and adds bo.
"""
import numpy as np

B, T0, D, H = 2, 4096, 1024, 16
DK = D // H          # 64
NCORES = 8
HPC = H // 4         # 4 heads per core
CW = HPC * DK        # 256 head-columns per core

_BUILD_CACHE = {}


def _build(T):
    import concourse.bacc as bacc
    import concourse.mybir as mybir
    import concourse.tile as tile
    from contextlib import ExitStack

    F32 = mybir.dt.float32
    F32R = mybir.dt.float32r
    BF16 = mybir.dt.bfloat16
    EXP = mybir.ActivationFunctionType.Exp

    NT = T // 512    # q-blocks of 512
    NJ = T // 128    # k-blocks of 128
    VTW = NJ * 384 + 64   # per j: ones|v0|v1|ones2|v2|v3, plus final ones

    nc = bacc.Bacc("TRN2", target_bir_lowering=False, debug=False, num_devices=8)

    xt_d = nc.dram_tensor("xt", [D, T], F32R, kind="ExternalInput")
    wq_d = nc.dram_tensor("wq", [128, 8 * 256], F32R, kind="ExternalInput")
    wk_d = nc.dram_tensor("wk", [128, 8 * 256], F32R, kind="ExternalInput")
    wv_d = nc.dram_tensor("wv", [128, 8 * 256], F32R, kind="ExternalInput")
    wo_d = nc.dram_tensor("wo", [128, 2 * 1024], F32R, kind="ExternalInput")
    bqc_d = nc.dram_tensor("bqc", [128, 2], F32, kind="ExternalInput")
    bkc_d = nc.dram_tensor("bkc", [128, 2], F32, kind="ExternalInput")
    bvb_d = nc.dram_tensor("bvb", [128, 256], F32, kind="ExternalInput")
    e2_d = nc.dram_tensor("e2sel", [128, 128], F32R, kind="ExternalInput")
    vpad_d = nc.dram_tensor("vpad", [128, NJ * 64], BF16, kind="ExternalInput")
    mtri_d = nc.dram_tensor("mtri", [128, 1024], BF16, kind="ExternalInput")
    ot_d = nc.dram_tensor("ot", [D, T], F32, kind="ExternalOutput")

    with tile.TileContext(nc) as tc, ExitStack() as ctx:
        ctx.enter_context(nc.allow_low_precision(reason="bf16/fp32r by design"))

        # ---- persistent SBUF ----
        per = ctx.enter_context(tc.tile_pool(name="persist", bufs=1))
        qt = [per.tile([128, T], BF16, name=f"qt{p}", tag=f"qt{p}") for p in range(2)]
        kta = [per.tile([128, T], BF16, name=f"kta{p}", tag=f"kta{p}") for p in range(2)]
        ktb = [per.tile([128, T], BF16, name=f"ktb{p}", tag=f"ktb{p}") for p in range(2)]
        vt = per.tile([128, VTW], BF16, name="vt", tag="vt")
        wq_sb = per.tile([128, 2048], F32R, name="wq", tag="wq")
        wk_sb = per.tile([128, 2048], F32R, name="wk", tag="wk")
        wv_sb = per.tile([128, 2048], F32R, name="wv", tag="wv")
        wo_sb = per.tile([128, 2048], F32R, name="wo", tag="wo")
        e2_sb = per.tile([128, 128], F32R, name="e2", tag="e2")
        mtri_sb = per.tile([128, 1024], BF16, name="mtri", tag="mtri")
        bqc_sb = per.tile([128, 2], F32, name="bqc", tag="bqc")
        bkc_sb = per.tile([128, 2], F32, name="bkc", tag="bkc")
        bvb_sb = per.tile([128, 256], F32, name="bvb", tag="bvb")
        cx = [per.tile([128, T], F32R, name=f"cx{p}", tag=f"cx{p}") for p in range(2)]
        zr = [per.tile([128, 512], F32R, name=f"zr{p}", tag=f"zr{p}") for p in range(2)]
        rr = [per.tile([128, 512], F32, name=f"rr{p}", tag=f"rr{p}") for p in range(2)]

        # ---- persistent PSUM (8 banks, region-aliased across phases) ----
        ps = ctx.enter_context(tc.tile_pool(name="ps", bufs=1, space="PSUM"))
        stp = [ps.tile([128, 1024], F32, name=f"stp{p}", tag=f"stp{p}")
               for p in range(2)]
        ua = [ps.tile([128, 512], F32, name=f"ua{p}", tag=f"ua{p}") for p in range(2)]
        ub = [ps.tile([128, 512], F32, name=f"ub{p}", tag=f"ub{p}") for p in range(2)]

        # ---- initial DMAs, ordered so proj(0) can start ASAP ----
        for db in range(8):
            nc.sync.dma_start(
                wq_sb[:, db * 256:(db + 1) * 256],
                wq_d.ap()[:, db * 256:(db + 1) * 256],
            )
        xpool = ctx.enter_context(tc.tile_pool(name="xts", bufs=16))

        def load_x(tb):
            xts = []
            for db in range(8):
                xtile = xpool.tile([128, 512], F32R, name="xt", tag="xt")
                nc.sync.dma_start(
                    xtile[:],
                    xt_d.ap()[db * 128:(db + 1) * 128, tb * 512:(tb + 1) * 512],
                )
                xts.append(xtile)
            return xts

        xts_cur = load_x(0)
        nc.sync.dma_start(wk_sb[:], wk_d.ap()[:])
        nc.sync.dma_start(wv_sb[:], wv_d.ap()[:])
        nc.sync.dma_start(bqc_sb[:], bqc_d.ap()[:])
        nc.sync.dma_start(bkc_sb[:], bkc_d.ap()[:])
        nc.sync.dma_start(bvb_sb[:], bvb_d.ap()[:])
        nc.sync.dma_start(e2_sb[:], e2_d.ap()[:])
        nc.sync.dma_start(mtri_sb[:], mtri_d.ap()[:])
        # ones/zero pad columns of the v-tiles (col 64 and 256 of each j blk)
        vt3 = vt[:, 0:NJ * 384].rearrange("p (j c) -> p j c", c=384)
        vsrc = vpad_d.ap()[:].rearrange("p (j c) -> p j c", c=64)
        nc.sync.dma_start(vt3[:, :, 0:64], vsrc)
        nc.sync.dma_start(vt3[:, :, 192:256], vsrc)
        nc.sync.dma_start(vt[:, NJ * 384: NJ * 384 + 64],
                          vpad_d.ap()[:, 0:64])
        nc.sync.dma_start(wo_sb[:], wo_d.ap()[:])
        # zero halves of kt tiles + zr scratch (once)
        for p in range(2):
            nc.vector.memset(kta[p][64:128, :], 0.0)
            nc.vector.memset(ktb[p][0:64, :], 0.0)

        opool = ctx.enter_context(tc.tile_pool(name="otile", bufs=4))
        epool = ctx.enter_context(tc.tile_pool(name="expt", bufs=4))

        mtri3 = mtri_sb[:].rearrange("p (h w) -> p h w", h=2)

        def proj_mm(out_ps, w_sb, p, xts):
            for db in range(8):
                nc.tensor.matmul(
                    out_ps,
                    w_sb[:, db * 256 + p * 128: db * 256 + (p + 1) * 128],
                    xts[db][:],
                    start=(db == 0), stop=(db == 7),
                )

        def norm_part1(tb1):
            # Z rows out of PSUM: Z_a -> zr row 0, Z_b -> zr row 64
            for p in range(2):
                nc.vector.tensor_copy(zr[p][0:64, :], ua[p][0:64, :])
                nc.vector.tensor_copy(zr[p][64:128, :], ub[p][64:128, :])

        def norm_rb(p, region):
            # e2 matmul: rb rows 0:64 <- Z_b (zr row 64), rows 64:128 <- Z_a
            nc.tensor.matmul(region, e2_sb[:], zr[p][:], start=True, stop=True)

        def norm_part2(p, region):
            nc.vector.reciprocal_approx_fast(out=rr[p][:], in_=region)

        def norm_muls(p, tb1):
            nc.vector.tensor_mul(
                cx[p][0:64, tb1 * 512:(tb1 + 1) * 512],
                ua[p][64:128, :], rr[p][64:128, :],
            )
            nc.vector.tensor_mul(
                cx[p][64:128, tb1 * 512:(tb1 + 1) * 512],
                ub[p][0:64, :], rr[p][0:64, :],
            )

        def oproj(tb1):
            for ob in range(8):
                po = (ua[0] if ob % 2 == 0 else ub[0])[:]
                nc.tensor.matmul(
                    po,
                    wo_sb[:, ob * 128:(ob + 1) * 128],
                    cx[0][:, tb1 * 512:(tb1 + 1) * 512],
                    start=True, stop=False, skip_group_check=True,
                )
                nc.tensor.matmul(
                    po,
                    wo_sb[:, 1024 + ob * 128: 1024 + (ob + 1) * 128],
                    cx[1][:, tb1 * 512:(tb1 + 1) * 512],
                    start=False, stop=True, skip_group_check=True,
                )
                ot_t = opool.tile([128, 512], F32, name="ot", tag="ot")
                nc.vector.tensor_copy(ot_t[:], po)
                nc.sync.dma_start(
                    ot_d.ap()[ob * 128:(ob + 1) * 128, tb1 * 512:(tb1 + 1) * 512],
                    ot_t[:],
                )

        for tb in range(NT):
            xts = xts_cur
            if tb + 1 < NT:
                xts_cur = load_x(tb + 1)

            # ---------- projections for tb (+ norm(tb-1) interleaved) ----------
            # psq(p0) -> stp0[:, 0:512]
            proj_mm(stp[0][:, 0:512], wq_sb, 0, xts)
            nc.vector.tensor_scalar_add(
                qt[0][:, tb * 512:(tb + 1) * 512], stp[0][:, 0:512],
                bqc_sb[:, 0:1],
            )
            if tb > 0:
                norm_part1(tb - 1)
            # psq(p1) -> stp0[:, 512:1024]
            proj_mm(stp[0][:, 512:1024], wq_sb, 1, xts)
            nc.vector.tensor_scalar_add(
                qt[1][:, tb * 512:(tb + 1) * 512], stp[0][:, 512:1024],
                bqc_sb[:, 1:2],
            )
            if tb > 0:
                norm_rb(0, stp[0][:, 0:512])
                norm_part2(0, stp[0][:, 0:512])
                norm_muls(0, tb - 1)
            # psk(p0) -> stp1[:, 0:512]
            proj_mm(stp[1][:, 0:512], wk_sb, 0, xts)
            nc.vector.tensor_scalar_add(
                kta[0][0:64, tb * 512:(tb + 1) * 512], stp[1][0:64, 0:512],
                bkc_sb[0:64, 0:1],
            )
            nc.vector.tensor_scalar_add(
                ktb[0][64:128, tb * 512:(tb + 1) * 512], stp[1][64:128, 0:512],
                bkc_sb[64:128, 0:1],
            )
            if tb > 0:
                norm_rb(1, stp[0][:, 512:1024])
                norm_part2(1, stp[0][:, 512:1024])
                norm_muls(1, tb - 1)
            # psk(p1) -> stp1[:, 512:1024]
            proj_mm(stp[1][:, 512:1024], wk_sb, 1, xts)
            nc.vector.tensor_scalar_add(
                kta[1][0:64, tb * 512:(tb + 1) * 512], stp[1][0:64, 512:1024],
                bkc_sb[0:64, 1:2],
            )
            nc.vector.tensor_scalar_add(
                ktb[1][64:128, tb * 512:(tb + 1) * 512], stp[1][64:128, 512:1024],
                bkc_sb[64:128, 1:2],
            )
            # psv: 4 sub-blocks of 128 t-rows -> ua0/ub0/ua1/ub1 [:, 0:256]
            psv_slots = [ua[0], ub[0], ua[1], ub[1]]
            for sub in range(4):
                j = tb * 4 + sub
                psv_t = psv_slots[sub]
                for db in range(8):
                    nc.tensor.matmul(
                        psv_t[:, 0:256],
                        xts[db][:, sub * 128:(sub + 1) * 128],
                        wv_sb[:, db * 256:(db + 1) * 256],
                        start=(db == 0), stop=(db == 7),
                    )
                # scatter v (+bias): v0|v1 -> [64:192], v2|v3 -> [256:384]
                nc.vector.tensor_add(
                    vt[:, j * 384 + 64: j * 384 + 192],
                    psv_t[:, 0:128], bvb_sb[:, 0:128],
                )
                nc.vector.tensor_add(
                    vt[:, j * 384 + 256: j * 384 + 384],
                    psv_t[:, 128:256], bvb_sb[:, 128:256],
                )

            # ---------- output projection for tb-1 ----------
            if tb > 0:
                oproj(tb - 1)

            # ---------- SDPA for i = tb ----------
            i = tb
            jmax = 4 * i + 4
            ets = {}
            for j in range(jmax):
                t = j - 4 * i
                cs = 128 * t if t >= 0 else 0
                w = 512 - cs
                for p in range(2):
                    nc.tensor.matmul(
                        stp[p][:, cs:512],
                        kta[p][:, j * 128:(j + 1) * 128],
                        qt[p][:, i * 512 + cs:(i + 1) * 512],
                        start=True, stop=True,
                    )
                    nc.tensor.matmul(
                        stp[p][:, 512 + cs:1024],
                        ktb[p][:, j * 128:(j + 1) * 128],
                        qt[p][:, i * 512 + cs:(i + 1) * 512],
                        start=True, stop=True,
                    )
                    et = epool.tile([128, 1024], BF16, name="et", tag="et")
                    src = stp[p][:].rearrange("p (h w) -> p h w", h=2)[:, :, cs:512]
                    dst = et[:].rearrange("p (h w) -> p h w", h=2)[:, :, cs:512]
                    nc.scalar.activation(dst, src, EXP, scale=0.125)
                    if t >= 0:
                        nc.vector.tensor_mul(dst, dst, mtri3[:, :, 0:w])
                    ets[(j, p)] = et
                if j >= 1:
                    _pv(nc, vt, ua, ub, ets, j - 1, jmax, 4 * i)
                    del ets[(j - 1, 0)], ets[(j - 1, 1)]
            _pv(nc, vt, ua, ub, ets, jmax - 1, jmax, 4 * i)

        # tail: norm + oproj for last block
        tb1 = NT - 1
        norm_part1(tb1)
        norm_rb(0, stp[0][:, 0:512])
        norm_part2(0, stp[0][:, 0:512])
        norm_muls(0, tb1)
        norm_rb(1, stp[0][:, 512:1024])
        norm_part2(1, stp[0][:, 512:1024])
        norm_muls(1, tb1)
        oproj(tb1)

    nc.compile()
    return nc


def _pv(nc, vt, ua, ub, ets, j, jmax, i4):
    t = j - i4
    cs = 128 * t if t >= 0 else 0
    st_flags = dict(start=(j == 0), stop=(j == jmax - 1), skip_group_check=True)
    for p in range(2):
        et = ets[(j, p)]
        base = j * 384 + 192 * p
        nc.tensor.matmul(
            ua[p][:, cs:512], vt[:, base: base + 128], et[:, cs:512], **st_flags
        )
        nc.tensor.matmul(
            ub[p][:, cs:512], vt[:, base + 128: base + 256],
            et[:, 512 + cs:1024], **st_flags
        )


def _get_built(T):
    if T not in _BUILD_CACHE:
        _BUILD_CACHE[T] = _build(T)
    return _BUILD_CACHE[T]


def _rearr_w(w):  # [1024, 256] -> [128, 8*256] (d-block major free dim)
    return np.ascontiguousarray(
        w.reshape(8, 128, 256).transpose(1, 0, 2).reshape(128, 8 * 256)
    )


def _numpy_ref(x, mask, Wq, bq, Wk, bk, Wv, bv, Wo, bo):
    T = x.shape[1]
    q = (x @ Wq + bq).reshape(B, T, H, DK).transpose(0, 2, 1, 3)
    k = (x @ Wk + bk).reshape(B, T, H, DK).transpose(0, 2, 1, 3)
    v = (x @ Wv + bv).reshape(B, T, H, DK).transpose(0, 2, 1, 3)
    s = np.einsum("bhqd,bhkd->bhqk", q, k) / np.sqrt(np.float32(DK))
    s = np.where(mask, s, s - 1e9)
    s = s - s.max(axis=-1, keepdims=True)
    e = np.exp(s)
    p = e / e.sum(axis=-1, keepdims=True)
    o = np.einsum("bhqk,bhkd->bhqd", p, v).transpose(0, 2, 1, 3).reshape(B, T, D)
    return (o @ Wo + bo).astype(np.float32)


def kernel(x, mask, Wq, bq, Wk, bk, Wv, bv, Wo, bo):
    from concourse import bass_utils

    x = np.ascontiguousarray(np.asarray(x, dtype=np.float32))
    mask = np.asarray(mask)
    T = x.shape[1]

    causal = bool(
        np.array_equal(mask[0, 0], np.tril(np.ones((T, T), dtype=bool)))
    )
    if not causal or x.shape != (B, T, D) or T % 512 != 0:
        return _numpy_ref(
            np.asarray(x, np.float32), mask,
            np.asarray(Wq, np.float32), np.asarray(bq, np.float32),
            np.asarray(Wk, np.float32), np.asarray(bk, np.float32),
            np.asarray(Wv, np.float32), np.asarray(bv, np.float32),
            np.asarray(Wo, np.float32), np.asarray(bo, np.float32),
        )

    in_maps = _make_in_maps(dict(x=x, Wq=Wq, bq=bq, Wk=Wk, bk=bk,
                                 Wv=Wv, bv=bv, Wo=Wo))
    nc = _get_built(T)
    res = bass_utils.run_bass_kernel_spmd(nc, in_maps, core_ids=list(range(NCORES)))

    out = np.zeros((B, T, D), np.float32)
    for c in range(NCORES):
        out[c // 4] += res.results[c]["ot"].T
    out += np.asarray(bo, np.float32)
    return out


def _make_in_maps(inputs):
    import ml_dtypes

    x = np.ascontiguousarray(np.asarray(inputs["x"], np.float32))
    T = x.shape[1]
    NJ = T // 128
    Wq = np.asarray(inputs["Wq"], np.float32)
    Wk = np.asarray(inputs["Wk"], np.float32)
    Wv = np.asarray(inputs["Wv"], np.float32)
    Wo = np.asarray(inputs["Wo"], np.float32)
    bq = np.asarray(inputs["bq"], np.float32)
    bk = np.asarray(inputs["bk"], np.float32)
    bv = np.asarray(inputs["bv"], np.float32)

    e2 = np.zeros((128, 128), np.float32)
    e2[64, 0:64] = 1.0
    e2[0, 64:128] = 1.0
    vpad = np.zeros((128, NJ * 64), ml_dtypes.bfloat16)
    vpad[:, ::64] = 1.0
    mtri = np.zeros((128, 1024), ml_dtypes.bfloat16)
    tri = (np.arange(128)[:, None] <= np.arange(512)[None, :])
    mtri[:, 0:512] = tri
    mtri[:, 512:1024] = tri

    xts = [np.ascontiguousarray(x[b].T) for b in range(B)]

    in_maps = []
    for c in range(NCORES):
        b, g = divmod(c, 4)
        cols = slice(g * CW, (g + 1) * CW)
        rows = slice(g * CW, (g + 1) * CW)
        wo_g = Wo[rows]  # [256, 1024]
        in_maps.append({
            "xt": xts[b],
            "wq": _rearr_w(Wq[:, cols]),
            "wk": _rearr_w(Wk[:, cols]),
            "wv": _rearr_w(Wv[:, cols]),
            "wo": np.ascontiguousarray(
                wo_g.reshape(2, 128, 1024).transpose(1, 0, 2).reshape(128, 2048)
            ),
            "bqc": np.ascontiguousarray(bq[cols].reshape(2, 128).T),
            "bkc": np.ascontiguousarray(bk[cols].reshape(2, 128).T),
            "bvb": np.ascontiguousarray(
                np.broadcast_to(bv[cols][None, :], (128, 256)).copy()
            ),
            "e2sel": e2,
            "vpad": vpad,
            "mtri": mtri,
        })

    return in_maps


# revision 3
# speedup vs baseline: 1.0128x; 1.0128x over previous
"""Multi-head causal attention (B=2, T=4096, D=1024, H=16) on 8 trn2 cores.

Sharding: core c = 4*b + g handles batch b and head-group g (4 heads).
Merged single-pass pipeline per 512-col q-block tb:
  proj(tb) -> norm(tb-1) -> oproj(tb-1) -> SDPA(i=tb)
SDPA streams in bf16 (q/k/v/probs); Z rows fused into PV via [v|ones]
stationary tiles; causal mask via DVE multiply with a triangular bf16
constant. Host sums the per-core partial O^T and adds bo.
"""
import numpy as np

B, T0, D, H = 2, 4096, 1024, 16
DK = D // H          # 64
NCORES = 8
HPC = H // 4         # 4 heads per core
CW = HPC * DK        # 256 head-columns per core

_BUILD_CACHE = {}


def _build(T):
    import concourse.bacc as bacc
    import concourse.mybir as mybir
    import concourse.tile as tile
    from contextlib import ExitStack

    F32 = mybir.dt.float32
    F32R = mybir.dt.float32r
    BF16 = mybir.dt.bfloat16
    EXP = mybir.ActivationFunctionType.Exp

    NT = T // 512    # q-blocks of 512
    NJ = T // 128    # k-blocks of 128
    VTW = NJ * 384 + 64   # per j: ones|v0|v1|ones2|v2|v3, plus final ones

    nc = bacc.Bacc("TRN2", target_bir_lowering=False, debug=False, num_devices=8)

    xt_d = nc.dram_tensor("xt", [D, T], F32R, kind="ExternalInput")
    wq_d = nc.dram_tensor("wq", [128, 8 * 256], F32R, kind="ExternalInput")
    wk_d = nc.dram_tensor("wk", [128, 8 * 256], F32R, kind="ExternalInput")
    wv_d = nc.dram_tensor("wv", [128, 8 * 256], F32R, kind="ExternalInput")
    wo_d = nc.dram_tensor("wo", [128, 2 * 1024], F32R, kind="ExternalInput")
    bqc_d = nc.dram_tensor("bqc", [128, 2], F32, kind="ExternalInput")
    bkc_d = nc.dram_tensor("bkc", [128, 2], F32, kind="ExternalInput")
    bvb_d = nc.dram_tensor("bvb", [128, 256], F32, kind="ExternalInput")
    e2_d = nc.dram_tensor("e2sel", [128, 128], F32R, kind="ExternalInput")
    vpad_d = nc.dram_tensor("vpad", [128, NJ * 64], BF16, kind="ExternalInput")
    mtri_d = nc.dram_tensor("mtri", [128, 1024], BF16, kind="ExternalInput")
    ot_d = nc.dram_tensor("ot", [D, T], F32, kind="ExternalOutput")

    with tile.TileContext(nc) as tc, ExitStack() as ctx:
        ctx.enter_context(nc.allow_low_precision(reason="bf16/fp32r by design"))

        # ---- persistent SBUF ----
        per = ctx.enter_context(tc.tile_pool(name="persist", bufs=1))
        qt = [per.tile([128, T], BF16, name=f"qt{p}", tag=f"qt{p}") for p in range(2)]
        kta = [per.tile([128, T], BF16, name=f"kta{p}", tag=f"kta{p}") for p in range(2)]
        ktb = [per.tile([128, T], BF16, name=f"ktb{p}", tag=f"ktb{p}") for p in range(2)]
        vt = per.tile([128, VTW], BF16, name="vt", tag="vt")
        wq_sb = per.tile([128, 2048], F32R, name="wq", tag="wq")
        wk_sb = per.tile([128, 2048], F32R, name="wk", tag="wk")
        wv_sb = per.tile([128, 2048], F32R, name="wv", tag="wv")
        wo_sb = per.tile([128, 2048], F32R, name="wo", tag="wo")
        e2_sb = per.tile([128, 128], F32R, name="e2", tag="e2")
        mtri_sb = per.tile([128, 1024], BF16, name="mtri", tag="mtri")
        bqc_sb = per.tile([128, 2], F32, name="bqc", tag="bqc")
        bkc_sb = per.tile([128, 2], F32, name="bkc", tag="bkc")
        bvb_sb = per.tile([128, 256], F32, name="bvb", tag="bvb")
        cx = [per.tile([128, T], F32R, name=f"cx{p}", tag=f"cx{p}") for p in range(2)]
        zr = [per.tile([128, 512], F32R, name=f"zr{p}", tag=f"zr{p}") for p in range(2)]
        rr = [per.tile([128, 512], F32, name=f"rr{p}", tag=f"rr{p}") for p in range(2)]

        # ---- persistent PSUM (8 banks, region-aliased across phases) ----
        ps = ctx.enter_context(tc.tile_pool(name="ps", bufs=1, space="PSUM"))
        stp = [ps.tile([128, 1024], F32, name=f"stp{p}", tag=f"stp{p}")
               for p in range(2)]
        ua = [ps.tile([128, 512], F32, name=f"ua{p}", tag=f"ua{p}") for p in range(2)]
        ub = [ps.tile([128, 512], F32, name=f"ub{p}", tag=f"ub{p}") for p in range(2)]

        # ---- initial DMAs, ordered so proj(0) can start ASAP ----
        xpool = ctx.enter_context(tc.tile_pool(name="xts", bufs=16))

        def load_x(tb, wq_interleave=False):
            xts = []
            for db in range(8):
                if wq_interleave:
                    nc.sync.dma_start(
                        wq_sb[:, db * 256:(db + 1) * 256],
                        wq_d.ap()[:, db * 256:(db + 1) * 256],
                    )
                xtile = xpool.tile([128, 512], F32R, name="xt", tag="xt")
                nc.sync.dma_start(
                    xtile[:],
                    xt_d.ap()[db * 128:(db + 1) * 128, tb * 512:(tb + 1) * 512],
                )
                xts.append(xtile)
            return xts

        xts_cur = load_x(0, wq_interleave=True)
        nc.sync.dma_start(wk_sb[:], wk_d.ap()[:])
        nc.sync.dma_start(wv_sb[:], wv_d.ap()[:])
        nc.sync.dma_start(bqc_sb[:], bqc_d.ap()[:])
        nc.sync.dma_start(bkc_sb[:], bkc_d.ap()[:])
        nc.sync.dma_start(bvb_sb[:], bvb_d.ap()[:])
        nc.sync.dma_start(e2_sb[:], e2_d.ap()[:])
        nc.sync.dma_start(mtri_sb[:], mtri_d.ap()[:])
        # ones/zero pad columns of the v-tiles (col 64 and 256 of each j blk)
        vt3 = vt[:, 0:NJ * 384].rearrange("p (j c) -> p j c", c=384)
        vsrc = vpad_d.ap()[:].rearrange("p (j c) -> p j c", c=64)
        nc.sync.dma_start(vt3[:, :, 0:64], vsrc)
        nc.sync.dma_start(vt3[:, :, 192:256], vsrc)
        nc.sync.dma_start(vt[:, NJ * 384: NJ * 384 + 64],
                          vpad_d.ap()[:, 0:64])
        nc.sync.dma_start(wo_sb[:], wo_d.ap()[:])
        # zero halves of kt tiles + zr scratch (once)
        for p in range(2):
            nc.vector.memset(kta[p][64:128, :], 0.0)
            nc.vector.memset(ktb[p][0:64, :], 0.0)

        opool = ctx.enter_context(tc.tile_pool(name="otile", bufs=6))
        epool = ctx.enter_context(tc.tile_pool(name="expt", bufs=6))

        mtri3 = mtri_sb[:].rearrange("p (h w) -> p h w", h=2)

        def proj_mm(out_ps, w_sb, p, xts):
            for db in range(8):
                nc.tensor.matmul(
                    out_ps,
                    w_sb[:, db * 256 + p * 128: db * 256 + (p + 1) * 128],
                    xts[db][:],
                    start=(db == 0), stop=(db == 7),
                )

        def norm_part1(tb1, tail=False):
            # Z rows out of PSUM: Z_a -> zr row 0, Z_b -> zr row 64
            # (at the tail ACT is idle -> use it for the PSUM reads)
            eng = nc.scalar.copy if tail else nc.vector.tensor_copy
            for p in range(2):
                eng(zr[p][0:64, :], ua[p][0:64, :])
                eng(zr[p][64:128, :], ub[p][64:128, :])

        def norm_rb(p, region):
            # e2 matmul: rb rows 0:64 <- Z_b (zr row 64), rows 64:128 <- Z_a
            nc.tensor.matmul(region, e2_sb[:], zr[p][:], start=True, stop=True)

        def norm_part2(p, region):
            nc.vector.reciprocal_approx_fast(out=rr[p][:], in_=region)

        def norm_muls(p, tb1):
            nc.vector.tensor_mul(
                cx[p][0:64, tb1 * 512:(tb1 + 1) * 512],
                ua[p][64:128, :], rr[p][64:128, :],
            )
            nc.vector.tensor_mul(
                cx[p][64:128, tb1 * 512:(tb1 + 1) * 512],
                ub[p][0:64, :], rr[p][0:64, :],
            )

        def oproj(tb1, tail=False):
            slots = [ua[0], ub[0], ua[1], ub[1]] if tail else [ua[0], ub[0]]
            for ob in range(8):
                po = slots[ob % len(slots)][:]
                nc.tensor.matmul(
                    po,
                    wo_sb[:, ob * 128:(ob + 1) * 128],
                    cx[0][:, tb1 * 512:(tb1 + 1) * 512],
                    start=True, stop=False, skip_group_check=True,
                )
                nc.tensor.matmul(
                    po,
                    wo_sb[:, 1024 + ob * 128: 1024 + (ob + 1) * 128],
                    cx[1][:, tb1 * 512:(tb1 + 1) * 512],
                    start=False, stop=True, skip_group_check=True,
                )
                ot_t = opool.tile([128, 512], F32, name="ot", tag="ot")
                (nc.scalar.copy if tail else nc.vector.tensor_copy)(ot_t[:], po)
                nc.sync.dma_start(
                    ot_d.ap()[ob * 128:(ob + 1) * 128, tb1 * 512:(tb1 + 1) * 512],
                    ot_t[:],
                )

        for tb in range(NT):
            xts = xts_cur
            if tb + 1 < NT:
                xts_cur = load_x(tb + 1)

            # ---------- projections for tb (+ norm(tb-1) interleaved) ----------
            # psq(p0) -> stp0[:, 0:512]
            proj_mm(stp[0][:, 0:512], wq_sb, 0, xts)
            nc.vector.tensor_scalar_add(
                qt[0][:, tb * 512:(tb + 1) * 512], stp[0][:, 0:512],
                bqc_sb[:, 0:1],
            )
            if tb > 0:
                norm_part1(tb - 1)
            # psq(p1) -> stp0[:, 512:1024]
            proj_mm(stp[0][:, 512:1024], wq_sb, 1, xts)
            nc.vector.tensor_scalar_add(
                qt[1][:, tb * 512:(tb + 1) * 512], stp[0][:, 512:1024],
                bqc_sb[:, 1:2],
            )
            if tb > 0:
                norm_rb(0, stp[0][:, 0:512])
                norm_part2(0, stp[0][:, 0:512])
                norm_muls(0, tb - 1)
            # psk(p0) -> stp1[:, 0:512]
            proj_mm(stp[1][:, 0:512], wk_sb, 0, xts)
            nc.vector.tensor_scalar_add(
                kta[0][0:64, tb * 512:(tb + 1) * 512], stp[1][0:64, 0:512],
                bkc_sb[0:64, 0:1],
            )
            nc.vector.tensor_scalar_add(
                ktb[0][64:128, tb * 512:(tb + 1) * 512], stp[1][64:128, 0:512],
                bkc_sb[64:128, 0:1],
            )
            if tb > 0:
                norm_rb(1, stp[0][:, 512:1024])
                norm_part2(1, stp[0][:, 512:1024])
                norm_muls(1, tb - 1)
            # psk(p1) -> stp1[:, 512:1024]
            proj_mm(stp[1][:, 512:1024], wk_sb, 1, xts)
            nc.vector.tensor_scalar_add(
                kta[1][0:64, tb * 512:(tb + 1) * 512], stp[1][0:64, 512:1024],
                bkc_sb[0:64, 1:2],
            )
            nc.vector.tensor_scalar_add(
                ktb[1][64:128, tb * 512:(tb + 1) * 512], stp[1][64:128, 512:1024],
                bkc_sb[64:128, 1:2],
            )
            # psv: 4 sub-blocks of 128 t-rows -> ua0/ub0/ua1/ub1 [:, 0:256]
            psv_slots = [ua[0], ub[0], ua[1], ub[1]]
            for sub in range(4):
                j = tb * 4 + sub
                psv_t = psv_slots[sub]
                for db in range(8):
                    nc.tensor.matmul(
                        psv_t[:, 0:256],
                        xts[db][:, sub * 128:(sub + 1) * 128],
                        wv_sb[:, db * 256:(db + 1) * 256],
                        start=(db == 0), stop=(db == 7),
                    )
                # scatter v (+bias): v0|v1 -> [64:192], v2|v3 -> [256:384]
                nc.vector.tensor_add(
                    vt[:, j * 384 + 64: j * 384 + 192],
                    psv_t[:, 0:128], bvb_sb[:, 0:128],
                )
                nc.vector.tensor_add(
                    vt[:, j * 384 + 256: j * 384 + 384],
                    psv_t[:, 128:256], bvb_sb[:, 128:256],
                )

            # ---------- output projection for tb-1 ----------
            if tb > 0:
                oproj(tb - 1)

            # ---------- SDPA for i = tb ----------
            i = tb
            jmax = 4 * i + 4
            ets = {}
            for j in range(jmax):
                t = j - 4 * i
                cs = 128 * t if t >= 0 else 0
                w = 512 - cs
                for p in range(2):
                    nc.tensor.matmul(
                        stp[p][:, cs:512],
                        kta[p][:, j * 128:(j + 1) * 128],
                        qt[p][:, i * 512 + cs:(i + 1) * 512],
                        start=True, stop=True,
                    )
                    nc.tensor.matmul(
                        stp[p][:, 512 + cs:1024],
                        ktb[p][:, j * 128:(j + 1) * 128],
                        qt[p][:, i * 512 + cs:(i + 1) * 512],
                        start=True, stop=True,
                    )
                    et = epool.tile([128, 1024], BF16, name="et", tag="et")
                    src = stp[p][:].rearrange("p (h w) -> p h w", h=2)[:, :, cs:512]
                    dst = et[:].rearrange("p (h w) -> p h w", h=2)[:, :, cs:512]
                    nc.scalar.activation(dst, src, EXP, scale=0.125)
                    if t >= 0:
                        nc.vector.tensor_mul(dst, dst, mtri3[:, :, 0:w])
                    ets[(j, p)] = et
                if j >= 1:
                    _pv(nc, vt, ua, ub, ets, j - 1, jmax, 4 * i)
                    del ets[(j - 1, 0)], ets[(j - 1, 1)]
            _pv(nc, vt, ua, ub, ets, jmax - 1, jmax, 4 * i)

        # tail: norm + oproj for last block
        tb1 = NT - 1
        norm_part1(tb1, tail=True)
        norm_rb(0, stp[0][:, 0:512])
        norm_part2(0, stp[0][:, 0:512])
        norm_muls(0, tb1)
        norm_rb(1, stp[0][:, 512:1024])
        norm_part2(1, stp[0][:, 512:1024])
        norm_muls(1, tb1)
        oproj(tb1, tail=True)

    nc.compile()
    return nc


def _pv(nc, vt, ua, ub, ets, j, jmax, i4):
    t = j - i4
    cs = 128 * t if t >= 0 else 0
    st_flags = dict(start=(j == 0), stop=(j == jmax - 1), skip_group_check=True)
    for p in range(2):
        et = ets[(j, p)]
        base = j * 384 + 192 * p
        nc.tensor.matmul(
            ua[p][:, cs:512], vt[:, base: base + 128], et[:, cs:512], **st_flags
        )
        nc.tensor.matmul(
            ub[p][:, cs:512], vt[:, base + 128: base + 256],
            et[:, 512 + cs:1024], **st_flags
        )


def _get_built(T):
    if T not in _BUILD_CACHE:
        _BUILD_CACHE[T] = _build(T)
    return _BUILD_CACHE[T]


def _rearr_w(w):  # [1024, 256] -> [128, 8*256] (d-block major free dim)
    return np.ascontiguousarray(
        w.reshape(8, 128, 256).transpose(1, 0, 2).reshape(128, 8 * 256)
    )


def _numpy_ref(x, mask, Wq, bq, Wk, bk, Wv, bv, Wo, bo):
    T = x.shape[1]
    q = (x @ Wq + bq).reshape(B, T, H, DK).transpose(0, 2, 1, 3)
    k = (x @ Wk + bk).reshape(B, T, H, DK).transpose(0, 2, 1, 3)
    v = (x @ Wv + bv).reshape(B, T, H, DK).transpose(0, 2, 1, 3)
    s = np.einsum("bhqd,bhkd->bhqk", q, k) / np.sqrt(np.float32(DK))
    s = np.where(mask, s, s - 1e9)
    s = s - s.max(axis=-1, keepdims=True)
    e = np.exp(s)
    p = e / e.sum(axis=-1, keepdims=True)
    o = np.einsum("bhqk,bhkd->bhqd", p, v).transpose(0, 2, 1, 3).reshape(B, T, D)
    return (o @ Wo + bo).astype(np.float32)


def kernel(x, mask, Wq, bq, Wk, bk, Wv, bv, Wo, bo):
    from concourse import bass_utils

    x = np.ascontiguousarray(np.asarray(x, dtype=np.float32))
    mask = np.asarray(mask)
    T = x.shape[1]

    causal = bool(
        np.array_equal(mask[0, 0], np.tril(np.ones((T, T), dtype=bool)))
    )
    if not causal or x.shape != (B, T, D) or T % 512 != 0:
        return _numpy_ref(
            np.asarray(x, np.float32), mask,
            np.asarray(Wq, np.float32), np.asarray(bq, np.float32),
            np.asarray(Wk, np.float32), np.asarray(bk, np.float32),
            np.asarray(Wv, np.float32), np.asarray(bv, np.float32),
            np.asarray(Wo, np.float32), np.asarray(bo, np.float32),
        )

    in_maps = _make_in_maps(dict(x=x, Wq=Wq, bq=bq, Wk=Wk, bk=bk,
                                 Wv=Wv, bv=bv, Wo=Wo))
    nc = _get_built(T)
    res = bass_utils.run_bass_kernel_spmd(nc, in_maps, core_ids=list(range(NCORES)))

    out = np.zeros((B, T, D), np.float32)
    for c in range(NCORES):
        out[c // 4] += res.results[c]["ot"].T
    out += np.asarray(bo, np.float32)
    return out


def _make_in_maps(inputs):
    import ml_dtypes

    x = np.ascontiguousarray(np.asarray(inputs["x"], np.float32))
    T = x.shape[1]
    NJ = T // 128
    Wq = np.asarray(inputs["Wq"], np.float32)
    Wk = np.asarray(inputs["Wk"], np.float32)
    Wv = np.asarray(inputs["Wv"], np.float32)
    Wo = np.asarray(inputs["Wo"], np.float32)
    bq = np.asarray(inputs["bq"], np.float32)
    bk = np.asarray(inputs["bk"], np.float32)
    bv = np.asarray(inputs["bv"], np.float32)

    e2 = np.zeros((128, 128), np.float32)
    e2[64, 0:64] = 1.0
    e2[0, 64:128] = 1.0
    vpad = np.zeros((128, NJ * 64), ml_dtypes.bfloat16)
    vpad[:, ::64] = 1.0
    mtri = np.zeros((128, 1024), ml_dtypes.bfloat16)
    tri = (np.arange(128)[:, None] <= np.arange(512)[None, :])
    mtri[:, 0:512] = tri
    mtri[:, 512:1024] = tri

    xts = [np.ascontiguousarray(x[b].T) for b in range(B)]

    in_maps = []
    for c in range(NCORES):
        b, g = divmod(c, 4)
        cols = slice(g * CW, (g + 1) * CW)
        rows = slice(g * CW, (g + 1) * CW)
        wo_g = Wo[rows]  # [256, 1024]
        in_maps.append({
            "xt": xts[b],
            "wq": _rearr_w(Wq[:, cols]),
            "wk": _rearr_w(Wk[:, cols]),
            "wv": _rearr_w(Wv[:, cols]),
            "wo": np.ascontiguousarray(
                wo_g.reshape(2, 128, 1024).transpose(1, 0, 2).reshape(128, 2048)
            ),
            "bqc": np.ascontiguousarray(bq[cols].reshape(2, 128).T),
            "bkc": np.ascontiguousarray(bk[cols].reshape(2, 128).T),
            "bvb": np.ascontiguousarray(
                np.broadcast_to(bv[cols][None, :], (128, 256)).copy()
            ),
            "e2sel": e2,
            "vpad": vpad,
            "mtri": mtri,
        })

    return in_maps


# revision 4
# speedup vs baseline: 1.0286x; 1.0156x over previous
"""Multi-head causal attention (B=2, T=4096, D=1024, H=16) on 8 trn2 cores.

Sharding: core c = 4*b + g handles batch b and head-group g (4 heads).
Merged single-pass pipeline per 512-col q-block tb:
  proj(tb) -> norm(tb-1) -> oproj(tb-1) -> SDPA(i=tb)
SDPA streams in bf16 (q/k/v/probs); Z rows fused into PV via [v|ones]
stationary tiles; causal mask via DVE multiply with a triangular bf16
constant. Host sums the per-core partial O^T and adds bo.
"""
import numpy as np

B, T0, D, H = 2, 4096, 1024, 16
DK = D // H          # 64
NCORES = 8
HPC = H // 4         # 4 heads per core
CW = HPC * DK        # 256 head-columns per core

_BUILD_CACHE = {}


def _build(T):
    import concourse.bacc as bacc
    import concourse.mybir as mybir
    import concourse.tile as tile
    from contextlib import ExitStack

    F32 = mybir.dt.float32
    F32R = mybir.dt.float32r
    BF16 = mybir.dt.bfloat16
    EXP = mybir.ActivationFunctionType.Exp

    NT = T // 512    # q-blocks of 512
    NJ = T // 128    # k-blocks of 128
    VTW = NJ * 384 + 64   # per j: ones|v0|v1|ones2|v2|v3, plus final ones

    nc = bacc.Bacc("TRN2", target_bir_lowering=False, debug=False, num_devices=8)

    xt_d = nc.dram_tensor("xt", [D, T], F32R, kind="ExternalInput")
    wq_d = nc.dram_tensor("wq", [128, 8 * 256], F32R, kind="ExternalInput")
    wk_d = nc.dram_tensor("wk", [128, 8 * 256], F32R, kind="ExternalInput")
    wv_d = nc.dram_tensor("wv", [128, 8 * 256], F32R, kind="ExternalInput")
    wo_d = nc.dram_tensor("wo", [128, 2 * 1024], F32R, kind="ExternalInput")
    bqc_d = nc.dram_tensor("bqc", [128, 2], F32, kind="ExternalInput")
    bkc_d = nc.dram_tensor("bkc", [128, 2], F32, kind="ExternalInput")
    bvb_d = nc.dram_tensor("bvb", [128, 256], F32, kind="ExternalInput")
    e2_d = nc.dram_tensor("e2sel", [128, 128], F32R, kind="ExternalInput")
    vpad_d = nc.dram_tensor("vpad", [128, NJ * 64], BF16, kind="ExternalInput")
    mtri_d = nc.dram_tensor("mtri", [128, 1024], BF16, kind="ExternalInput")
    ot_d = nc.dram_tensor("ot", [D, T], F32, kind="ExternalOutput")

    with tile.TileContext(nc) as tc, ExitStack() as ctx:
        ctx.enter_context(nc.allow_low_precision(reason="bf16/fp32r by design"))

        # ---- persistent SBUF ----
        per = ctx.enter_context(tc.tile_pool(name="persist", bufs=1))
        qt = [per.tile([128, T], BF16, name=f"qt{p}", tag=f"qt{p}") for p in range(2)]
        kta = [per.tile([128, T], BF16, name=f"kta{p}", tag=f"kta{p}") for p in range(2)]
        ktb = [per.tile([128, T], BF16, name=f"ktb{p}", tag=f"ktb{p}") for p in range(2)]
        vt = per.tile([128, VTW], BF16, name="vt", tag="vt")
        wq_sb = per.tile([128, 2048], F32R, name="wq", tag="wq")
        wk_sb = per.tile([128, 2048], F32R, name="wk", tag="wk")
        wv_sb = per.tile([128, 2048], F32R, name="wv", tag="wv")
        wo_sb = per.tile([128, 2048], F32R, name="wo", tag="wo")
        e2_sb = per.tile([128, 128], F32R, name="e2", tag="e2")
        mtri_sb = per.tile([128, 1024], BF16, name="mtri", tag="mtri")
        bqc_sb = per.tile([128, 2], F32, name="bqc", tag="bqc")
        bkc_sb = per.tile([128, 2], F32, name="bkc", tag="bkc")
        bvb_sb = per.tile([128, 256], F32, name="bvb", tag="bvb")
        cx = [per.tile([128, T], F32R, name=f"cx{p}", tag=f"cx{p}") for p in range(2)]
        zr = [per.tile([128, 512], F32R, name=f"zr{p}", tag=f"zr{p}") for p in range(2)]
        rr = [per.tile([128, 512], F32, name=f"rr{p}", tag=f"rr{p}") for p in range(2)]

        # ---- persistent PSUM (8 banks, region-aliased across phases) ----
        ps = ctx.enter_context(tc.tile_pool(name="ps", bufs=1, space="PSUM"))
        stp = [ps.tile([128, 1024], F32, name=f"stp{p}", tag=f"stp{p}")
               for p in range(2)]
        ua = [ps.tile([128, 512], F32, name=f"ua{p}", tag=f"ua{p}") for p in range(2)]
        ub = [ps.tile([128, 512], F32, name=f"ub{p}", tag=f"ub{p}") for p in range(2)]

        # ---- initial DMAs, ordered so proj(0) can start ASAP ----
        xpool = ctx.enter_context(tc.tile_pool(name="xts", bufs=16))

        def load_x(tb, wq_interleave=False):
            xts = []
            for db in range(8):
                if wq_interleave:
                    nc.sync.dma_start(
                        wq_sb[:, db * 256:(db + 1) * 256],
                        wq_d.ap()[:, db * 256:(db + 1) * 256],
                    )
                xtile = xpool.tile([128, 512], F32R, name="xt", tag="xt")
                nc.sync.dma_start(
                    xtile[:],
                    xt_d.ap()[db * 128:(db + 1) * 128, tb * 512:(tb + 1) * 512],
                )
                xts.append(xtile)
            return xts

        xts_cur = load_x(0, wq_interleave=True)
        nc.sync.dma_start(wk_sb[:], wk_d.ap()[:])
        nc.sync.dma_start(wv_sb[:], wv_d.ap()[:])
        nc.sync.dma_start(bqc_sb[:], bqc_d.ap()[:])
        nc.sync.dma_start(bkc_sb[:], bkc_d.ap()[:])
        nc.sync.dma_start(bvb_sb[:], bvb_d.ap()[:])
        nc.sync.dma_start(e2_sb[:], e2_d.ap()[:])
        nc.sync.dma_start(mtri_sb[:], mtri_d.ap()[:])
        # ones/zero pad columns of the v-tiles (col 64 and 256 of each j blk)
        vt3 = vt[:, 0:NJ * 384].rearrange("p (j c) -> p j c", c=384)
        vsrc = vpad_d.ap()[:].rearrange("p (j c) -> p j c", c=64)
        nc.sync.dma_start(vt3[:, :, 0:64], vsrc)
        nc.sync.dma_start(vt3[:, :, 192:256], vsrc)
        nc.sync.dma_start(vt[:, NJ * 384: NJ * 384 + 64],
                          vpad_d.ap()[:, 0:64])
        nc.sync.dma_start(wo_sb[:], wo_d.ap()[:])
        # zero halves of kt tiles + zr scratch (once)
        for p in range(2):
            nc.vector.memset(kta[p][64:128, :], 0.0)
            nc.vector.memset(ktb[p][0:64, :], 0.0)

        opool = ctx.enter_context(tc.tile_pool(name="otile", bufs=6))
        epool = ctx.enter_context(tc.tile_pool(name="expt", bufs=6))

        mtri3 = mtri_sb[:].rearrange("p (h w) -> p h w", h=2)

        def proj_mm(out_ps, w_sb, p, xts):
            for db in range(8):
                nc.tensor.matmul(
                    out_ps,
                    w_sb[:, db * 256 + p * 128: db * 256 + (p + 1) * 128],
                    xts[db][:],
                    start=(db == 0), stop=(db == 7),
                )

        def norm_part1(tb1, tail=False):
            # Z rows out of PSUM: Z_a -> zr row 0, Z_b -> zr row 64
            # (at the tail ACT is idle -> use it for the PSUM reads)
            eng = nc.scalar.copy if tail else nc.vector.tensor_copy
            for p in range(2):
                eng(zr[p][0:64, :], ua[p][0:64, :])
                eng(zr[p][64:128, :], ub[p][64:128, :])

        def norm_rb(p, region):
            # e2 matmul: rb rows 0:64 <- Z_b (zr row 64), rows 64:128 <- Z_a
            nc.tensor.matmul(region, e2_sb[:], zr[p][:], start=True, stop=True)

        def norm_part2(p, region):
            nc.vector.reciprocal_approx_fast(out=rr[p][:], in_=region)

        def norm_muls(p, tb1):
            nc.vector.tensor_mul(
                cx[p][0:64, tb1 * 512:(tb1 + 1) * 512],
                ua[p][64:128, :], rr[p][64:128, :],
            )
            nc.vector.tensor_mul(
                cx[p][64:128, tb1 * 512:(tb1 + 1) * 512],
                ub[p][0:64, :], rr[p][0:64, :],
            )

        def oproj(tb1, tail=False):
            slots = [ua[0], ub[0], ua[1], ub[1]] if tail else [ua[0], ub[0]]
            for ob in range(8):
                po = slots[ob % len(slots)][:]
                nc.tensor.matmul(
                    po,
                    wo_sb[:, ob * 128:(ob + 1) * 128],
                    cx[0][:, tb1 * 512:(tb1 + 1) * 512],
                    start=True, stop=False, skip_group_check=True,
                )
                nc.tensor.matmul(
                    po,
                    wo_sb[:, 1024 + ob * 128: 1024 + (ob + 1) * 128],
                    cx[1][:, tb1 * 512:(tb1 + 1) * 512],
                    start=False, stop=True, skip_group_check=True,
                )
                ot_t = opool.tile([128, 512], F32, name="ot", tag="ot")
                (nc.scalar.copy if tail else nc.vector.tensor_copy)(ot_t[:], po)
                nc.sync.dma_start(
                    ot_d.ap()[ob * 128:(ob + 1) * 128, tb1 * 512:(tb1 + 1) * 512],
                    ot_t[:],
                )

        for tb in range(NT):
            xts = xts_cur
            if tb + 1 < NT:
                xts_cur = load_x(tb + 1)

            # ---------- projections for tb (+ norm(tb-1) interleaved) ----------
            # psq(p0) -> stp0[:, 0:512]
            proj_mm(stp[0][:, 0:512], wq_sb, 0, xts)
            nc.vector.tensor_scalar_add(
                qt[0][:, tb * 512:(tb + 1) * 512], stp[0][:, 0:512],
                bqc_sb[:, 0:1],
            )
            if tb > 0:
                norm_part1(tb - 1)
            # psq(p1) -> stp0[:, 512:1024]
            proj_mm(stp[0][:, 512:1024], wq_sb, 1, xts)
            nc.vector.tensor_scalar_add(
                qt[1][:, tb * 512:(tb + 1) * 512], stp[0][:, 512:1024],
                bqc_sb[:, 1:2],
            )
            if tb > 0:
                norm_rb(0, stp[0][:, 0:512])
                norm_part2(0, stp[0][:, 0:512])
                norm_muls(0, tb - 1)
            # psk(p0) -> stp1[:, 0:512]
            proj_mm(stp[1][:, 0:512], wk_sb, 0, xts)
            nc.vector.tensor_scalar_add(
                kta[0][0:64, tb * 512:(tb + 1) * 512], stp[1][0:64, 0:512],
                bkc_sb[0:64, 0:1],
            )
            nc.vector.tensor_scalar_add(
                ktb[0][64:128, tb * 512:(tb + 1) * 512], stp[1][64:128, 0:512],
                bkc_sb[64:128, 0:1],
            )
            if tb > 0:
                norm_rb(1, stp[0][:, 512:1024])
                norm_part2(1, stp[0][:, 512:1024])
                norm_muls(1, tb - 1)
            # psk(p1) -> stp1[:, 512:1024]
            proj_mm(stp[1][:, 512:1024], wk_sb, 1, xts)
            nc.vector.tensor_scalar_add(
                kta[1][0:64, tb * 512:(tb + 1) * 512], stp[1][0:64, 512:1024],
                bkc_sb[0:64, 1:2],
            )
            nc.vector.tensor_scalar_add(
                ktb[1][64:128, tb * 512:(tb + 1) * 512], stp[1][64:128, 512:1024],
                bkc_sb[64:128, 1:2],
            )
            # psv: 4 sub-blocks of 128 t-rows -> ua0/ub0/ua1/ub1 [:, 0:256]
            psv_slots = [ua[0], ub[0], ua[1], ub[1]]
            for sub in range(4):
                j = tb * 4 + sub
                psv_t = psv_slots[sub]
                for db in range(8):
                    nc.tensor.matmul(
                        psv_t[:, 0:256],
                        xts[db][:, sub * 128:(sub + 1) * 128],
                        wv_sb[:, db * 256:(db + 1) * 256],
                        start=(db == 0), stop=(db == 7),
                    )
                # scatter v (+bias): v0|v1 -> [64:192], v2|v3 -> [256:384]
                nc.vector.tensor_add(
                    vt[:, j * 384 + 64: j * 384 + 192],
                    psv_t[:, 0:128], bvb_sb[:, 0:128],
                )
                nc.vector.tensor_add(
                    vt[:, j * 384 + 256: j * 384 + 384],
                    psv_t[:, 128:256], bvb_sb[:, 128:256],
                )

            # ---------- output projection for tb-1 ----------
            if tb > 0:
                oproj(tb - 1)

            # ---------- SDPA for i = tb ----------
            i = tb
            jmax = 4 * i + 4
            ets = {}
            for j in range(jmax):
                t = j - 4 * i
                cs = 128 * t if t >= 0 else 0
                w = 512 - cs
                for p in range(2):
                    nc.tensor.matmul(
                        stp[p][:, cs:512],
                        kta[p][:, j * 128:(j + 1) * 128],
                        qt[p][:, i * 512 + cs:(i + 1) * 512],
                        start=True, stop=True,
                    )
                    nc.tensor.matmul(
                        stp[p][:, 512 + cs:1024],
                        ktb[p][:, j * 128:(j + 1) * 128],
                        qt[p][:, i * 512 + cs:(i + 1) * 512],
                        start=True, stop=True,
                    )
                    et = epool.tile([128, 1024], BF16, name="et", tag="et")
                    src = stp[p][:].rearrange("p (h w) -> p h w", h=2)[:, :, cs:512]
                    dst = et[:].rearrange("p (h w) -> p h w", h=2)[:, :, cs:512]
                    nc.scalar.activation(dst, src, EXP, scale=0.125)
                    if t >= 0:
                        nc.vector.tensor_mul(dst, dst, mtri3[:, :, 0:w])
                    ets[(j, p)] = et
                if j >= 2:
                    _pv(nc, vt, ua, ub, ets, j - 2, jmax, 4 * i)
                    del ets[(j - 2, 0)], ets[(j - 2, 1)]
            _pv(nc, vt, ua, ub, ets, jmax - 2, jmax, 4 * i)
            _pv(nc, vt, ua, ub, ets, jmax - 1, jmax, 4 * i)

        # tail: norm + oproj for last block
        tb1 = NT - 1
        norm_part1(tb1, tail=True)
        norm_rb(0, stp[0][:, 0:512])
        norm_part2(0, stp[0][:, 0:512])
        norm_muls(0, tb1)
        norm_rb(1, stp[0][:, 512:1024])
        norm_part2(1, stp[0][:, 512:1024])
        norm_muls(1, tb1)
        oproj(tb1, tail=True)

    nc.compile()
    return nc


def _pv(nc, vt, ua, ub, ets, j, jmax, i4):
    t = j - i4
    cs = 128 * t if t >= 0 else 0
    st_flags = dict(start=(j == 0), stop=(j == jmax - 1), skip_group_check=True)
    for p in range(2):
        et = ets[(j, p)]
        base = j * 384 + 192 * p
        nc.tensor.matmul(
            ua[p][:, cs:512], vt[:, base: base + 128], et[:, cs:512], **st_flags
        )
        nc.tensor.matmul(
            ub[p][:, cs:512], vt[:, base + 128: base + 256],
            et[:, 512 + cs:1024], **st_flags
        )


def _get_built(T):
    if T not in _BUILD_CACHE:
        _BUILD_CACHE[T] = _build(T)
    return _BUILD_CACHE[T]


def _rearr_w(w):  # [1024, 256] -> [128, 8*256] (d-block major free dim)
    return np.ascontiguousarray(
        w.reshape(8, 128, 256).transpose(1, 0, 2).reshape(128, 8 * 256)
    )


def _numpy_ref(x, mask, Wq, bq, Wk, bk, Wv, bv, Wo, bo):
    T = x.shape[1]
    q = (x @ Wq + bq).reshape(B, T, H, DK).transpose(0, 2, 1, 3)
    k = (x @ Wk + bk).reshape(B, T, H, DK).transpose(0, 2, 1, 3)
    v = (x @ Wv + bv).reshape(B, T, H, DK).transpose(0, 2, 1, 3)
    s = np.einsum("bhqd,bhkd->bhqk", q, k) / np.sqrt(np.float32(DK))
    s = np.where(mask, s, s - 1e9)
    s = s - s.max(axis=-1, keepdims=True)
    e = np.exp(s)
    p = e / e.sum(axis=-1, keepdims=True)
    o = np.einsum("bhqk,bhkd->bhqd", p, v).transpose(0, 2, 1, 3).reshape(B, T, D)
    return (o @ Wo + bo).astype(np.float32)


def kernel(x, mask, Wq, bq, Wk, bk, Wv, bv, Wo, bo):
    from concourse import bass_utils

    x = np.ascontiguousarray(np.asarray(x, dtype=np.float32))
    mask = np.asarray(mask)
    T = x.shape[1]

    causal = bool(
        np.array_equal(mask[0, 0], np.tril(np.ones((T, T), dtype=bool)))
    )
    if not causal or x.shape != (B, T, D) or T % 512 != 0:
        return _numpy_ref(
            np.asarray(x, np.float32), mask,
            np.asarray(Wq, np.float32), np.asarray(bq, np.float32),
            np.asarray(Wk, np.float32), np.asarray(bk, np.float32),
            np.asarray(Wv, np.float32), np.asarray(bv, np.float32),
            np.asarray(Wo, np.float32), np.asarray(bo, np.float32),
        )

    in_maps = _make_in_maps(dict(x=x, Wq=Wq, bq=bq, Wk=Wk, bk=bk,
                                 Wv=Wv, bv=bv, Wo=Wo))
    nc = _get_built(T)
    res = bass_utils.run_bass_kernel_spmd(nc, in_maps, core_ids=list(range(NCORES)))

    out = np.zeros((B, T, D), np.float32)
    for c in range(NCORES):
        out[c // 4] += res.results[c]["ot"].T
    out += np.asarray(bo, np.float32)
    return out


def _make_in_maps(inputs):
    import ml_dtypes

    x = np.ascontiguousarray(np.asarray(inputs["x"], np.float32))
    T = x.shape[1]
    NJ = T // 128
    Wq = np.asarray(inputs["Wq"], np.float32)
    Wk = np.asarray(inputs["Wk"], np.float32)
    Wv = np.asarray(inputs["Wv"], np.float32)
    Wo = np.asarray(inputs["Wo"], np.float32)
    bq = np.asarray(inputs["bq"], np.float32)
    bk = np.asarray(inputs["bk"], np.float32)
    bv = np.asarray(inputs["bv"], np.float32)

    e2 = np.zeros((128, 128), np.float32)
    e2[64, 0:64] = 1.0
    e2[0, 64:128] = 1.0
    vpad = np.zeros((128, NJ * 64), ml_dtypes.bfloat16)
    vpad[:, ::64] = 1.0
    mtri = np.zeros((128, 1024), ml_dtypes.bfloat16)
    tri = (np.arange(128)[:, None] <= np.arange(512)[None, :])
    mtri[:, 0:512] = tri
    mtri[:, 512:1024] = tri

    xts = [np.ascontiguousarray(x[b].T) for b in range(B)]

    in_maps = []
    for c in range(NCORES):
        b, g = divmod(c, 4)
        cols = slice(g * CW, (g + 1) * CW)
        rows = slice(g * CW, (g + 1) * CW)
        wo_g = Wo[rows]  # [256, 1024]
        in_maps.append({
            "xt": xts[b],
            "wq": _rearr_w(Wq[:, cols]),
            "wk": _rearr_w(Wk[:, cols]),
            "wv": _rearr_w(Wv[:, cols]),
            "wo": np.ascontiguousarray(
                wo_g.reshape(2, 128, 1024).transpose(1, 0, 2).reshape(128, 2048)
            ),
            "bqc": np.ascontiguousarray(bq[cols].reshape(2, 128).T),
            "bkc": np.ascontiguousarray(bk[cols].reshape(2, 128).T),
            "bvb": np.ascontiguousarray(
                np.broadcast_to(bv[cols][None, :], (128, 256)).copy()
            ),
            "e2sel": e2,
            "vpad": vpad,
            "mtri": mtri,
        })

    return in_maps


# revision 5
# speedup vs baseline: 1.0421x; 1.0132x over previous
"""Multi-head causal attention (B=2, T=4096, D=1024, H=16) on 8 trn2 cores.

Sharding: core c = 4*b + g handles batch b and head-group g (4 heads).
Merged single-pass pipeline per 512-col q-block tb:
  proj(tb) -> norm(tb-1) -> oproj(tb-1) -> SDPA(i=tb)
SDPA streams in bf16 (q/k/v/probs); Z rows fused into PV via [v|ones]
stationary tiles; causal mask via DVE multiply with a triangular bf16
constant. Host sums the per-core partial O^T and adds bo.
"""
import numpy as np

B, T0, D, H = 2, 4096, 1024, 16
DK = D // H          # 64
NCORES = 8
HPC = H // 4         # 4 heads per core
CW = HPC * DK        # 256 head-columns per core

_BUILD_CACHE = {}


def _build(T):
    import concourse.bacc as bacc
    import concourse.mybir as mybir
    import concourse.tile as tile
    from contextlib import ExitStack

    F32 = mybir.dt.float32
    F32R = mybir.dt.float32r
    BF16 = mybir.dt.bfloat16
    EXP = mybir.ActivationFunctionType.Exp

    NT = T // 512    # q-blocks of 512
    NJ = T // 128    # k-blocks of 128
    VTW = NJ * 384 + 64   # per j: ones|v0|v1|ones2|v2|v3, plus final ones

    nc = bacc.Bacc("TRN2", target_bir_lowering=False, debug=False, num_devices=8)

    xt_d = nc.dram_tensor("xt", [D, T], F32R, kind="ExternalInput")
    wq_d = nc.dram_tensor("wq", [128, 8 * 256], F32R, kind="ExternalInput")
    wk_d = nc.dram_tensor("wk", [128, 8 * 256], F32R, kind="ExternalInput")
    wv_d = nc.dram_tensor("wv", [128, 8 * 256], F32R, kind="ExternalInput")
    wo_d = nc.dram_tensor("wo", [128, 2 * 1024], F32R, kind="ExternalInput")
    bqc_d = nc.dram_tensor("bqc", [128, 2], F32, kind="ExternalInput")
    bkc_d = nc.dram_tensor("bkc", [128, 2], F32, kind="ExternalInput")
    bvb_d = nc.dram_tensor("bvb", [128, 256], F32, kind="ExternalInput")
    e2_d = nc.dram_tensor("e2sel", [128, 128], F32R, kind="ExternalInput")
    vpad_d = nc.dram_tensor("vpad", [128, NJ * 64], BF16, kind="ExternalInput")
    mtri_d = nc.dram_tensor("mtri", [128, 1024], BF16, kind="ExternalInput")
    ot_d = nc.dram_tensor("ot", [D, T], F32, kind="ExternalOutput")

    with tile.TileContext(nc) as tc, ExitStack() as ctx:
        ctx.enter_context(nc.allow_low_precision(reason="bf16/fp32r by design"))

        # ---- persistent SBUF ----
        per = ctx.enter_context(tc.tile_pool(name="persist", bufs=1))
        qt = [per.tile([128, T], BF16, name=f"qt{p}", tag=f"qt{p}") for p in range(2)]
        kta = [per.tile([128, T], BF16, name=f"kta{p}", tag=f"kta{p}") for p in range(2)]
        ktb = [per.tile([128, T], BF16, name=f"ktb{p}", tag=f"ktb{p}") for p in range(2)]
        vt = per.tile([128, VTW], BF16, name="vt", tag="vt")
        wq_sb = per.tile([128, 2048], F32R, name="wq", tag="wq")
        wk_sb = per.tile([128, 2048], F32R, name="wk", tag="wk")
        wv_sb = per.tile([128, 2048], F32R, name="wv", tag="wv")
        wo_sb = per.tile([128, 2048], F32R, name="wo", tag="wo")
        e2_sb = per.tile([128, 128], F32R, name="e2", tag="e2")
        mtri_sb = per.tile([128, 1024], BF16, name="mtri", tag="mtri")
        bqc_sb = per.tile([128, 2], F32, name="bqc", tag="bqc")
        bkc_sb = per.tile([128, 2], F32, name="bkc", tag="bkc")
        bvb_sb = per.tile([128, 256], F32, name="bvb", tag="bvb")
        cx = [per.tile([128, T], F32R, name=f"cx{p}", tag=f"cx{p}") for p in range(2)]
        zr = [per.tile([128, 512], F32R, name=f"zr{p}", tag=f"zr{p}") for p in range(2)]
        rr = [per.tile([128, 512], F32, name=f"rr{p}", tag=f"rr{p}") for p in range(2)]

        # ---- persistent PSUM (8 banks, region-aliased across phases) ----
        ps = ctx.enter_context(tc.tile_pool(name="ps", bufs=1, space="PSUM"))
        stp = [ps.tile([128, 1024], F32, name=f"stp{p}", tag=f"stp{p}")
               for p in range(2)]
        ua = [ps.tile([128, 512], F32, name=f"ua{p}", tag=f"ua{p}") for p in range(2)]
        ub = [ps.tile([128, 512], F32, name=f"ub{p}", tag=f"ub{p}") for p in range(2)]

        # ---- initial DMAs, ordered so proj(0) can start ASAP ----
        xpool = ctx.enter_context(tc.tile_pool(name="xts", bufs=16))

        def load_x(tb, wq_interleave=False):
            xts = []
            for db in range(8):
                if wq_interleave:
                    nc.sync.dma_start(
                        wq_sb[:, db * 256:(db + 1) * 256],
                        wq_d.ap()[:, db * 256:(db + 1) * 256],
                    )
                xtile = xpool.tile([128, 512], F32R, name="xt", tag="xt")
                nc.sync.dma_start(
                    xtile[:],
                    xt_d.ap()[db * 128:(db + 1) * 128, tb * 512:(tb + 1) * 512],
                )
                xts.append(xtile)
            return xts

        xts_cur = load_x(0, wq_interleave=True)
        nc.sync.dma_start(wk_sb[:], wk_d.ap()[:])
        nc.sync.dma_start(wv_sb[:], wv_d.ap()[:])
        nc.sync.dma_start(bqc_sb[:], bqc_d.ap()[:])
        nc.sync.dma_start(bkc_sb[:], bkc_d.ap()[:])
        nc.sync.dma_start(bvb_sb[:], bvb_d.ap()[:])
        nc.sync.dma_start(e2_sb[:], e2_d.ap()[:])
        nc.sync.dma_start(mtri_sb[:], mtri_d.ap()[:])
        # ones/zero pad columns of the v-tiles (col 64 and 256 of each j blk)
        vt3 = vt[:, 0:NJ * 384].rearrange("p (j c) -> p j c", c=384)
        vsrc = vpad_d.ap()[:].rearrange("p (j c) -> p j c", c=64)
        nc.sync.dma_start(vt3[:, :, 0:64], vsrc)
        nc.sync.dma_start(vt3[:, :, 192:256], vsrc)
        nc.sync.dma_start(vt[:, NJ * 384: NJ * 384 + 64],
                          vpad_d.ap()[:, 0:64])
        nc.sync.dma_start(wo_sb[:], wo_d.ap()[:])
        # zero halves of kt tiles + zr scratch (once)
        for p in range(2):
            nc.vector.memset(kta[p][64:128, :], 0.0)
            nc.vector.memset(ktb[p][0:64, :], 0.0)

        opool = ctx.enter_context(tc.tile_pool(name="otile", bufs=6))
        epool = ctx.enter_context(tc.tile_pool(name="expt", bufs=6))

        mtri3 = mtri_sb[:].rearrange("p (h w) -> p h w", h=2)

        def proj_mm(out_ps, w_sb, p, xts):
            for db in range(8):
                nc.tensor.matmul(
                    out_ps,
                    w_sb[:, db * 256 + p * 128: db * 256 + (p + 1) * 128],
                    xts[db][:],
                    start=(db == 0), stop=(db == 7),
                )

        def norm_part1(tb1, tail=False):
            # Z rows out of PSUM: Z_a -> zr row 0, Z_b -> zr row 64
            # (at the tail ACT is idle -> use it for the PSUM reads)
            eng = nc.scalar.copy if tail else nc.vector.tensor_copy
            for p in range(2):
                eng(zr[p][0:64, :], ua[p][0:64, :])
                eng(zr[p][64:128, :], ub[p][64:128, :])

        def norm_rb(p, region):
            # e2 matmul: rb rows 0:64 <- Z_b (zr row 64), rows 64:128 <- Z_a
            nc.tensor.matmul(region, e2_sb[:], zr[p][:], start=True, stop=True)

        def norm_part2(p, region):
            nc.vector.reciprocal_approx_fast(out=rr[p][:], in_=region)

        def norm_muls(p, tb1):
            nc.vector.tensor_mul(
                cx[p][0:64, tb1 * 512:(tb1 + 1) * 512],
                ua[p][64:128, :], rr[p][64:128, :],
            )
            nc.vector.tensor_mul(
                cx[p][64:128, tb1 * 512:(tb1 + 1) * 512],
                ub[p][0:64, :], rr[p][0:64, :],
            )

        def oproj(tb1, tail=False):
            slots = [ua[0], ub[0], ua[1], ub[1]] if tail else [ua[0], ub[0]]
            for ob in range(8):
                po = slots[ob % len(slots)][:]
                nc.tensor.matmul(
                    po,
                    wo_sb[:, ob * 128:(ob + 1) * 128],
                    cx[0][:, tb1 * 512:(tb1 + 1) * 512],
                    start=True, stop=False, skip_group_check=True,
                )
                nc.tensor.matmul(
                    po,
                    wo_sb[:, 1024 + ob * 128: 1024 + (ob + 1) * 128],
                    cx[1][:, tb1 * 512:(tb1 + 1) * 512],
                    start=False, stop=True, skip_group_check=True,
                )
                ot_t = opool.tile([128, 512], F32, name="ot", tag="ot")
                (nc.scalar.copy if tail else nc.vector.tensor_copy)(ot_t[:], po)
                nc.sync.dma_start(
                    ot_d.ap()[ob * 128:(ob + 1) * 128, tb1 * 512:(tb1 + 1) * 512],
                    ot_t[:],
                )

        for tb in range(NT):
            xts = xts_cur
            if tb + 1 < NT:
                xts_cur = load_x(tb + 1)

            # ---------- projections for tb (+ norm(tb-1) interleaved) ----------
            # psq(p0) -> stp0[:, 0:512]
            if tb > 0:
                norm_part1(tb - 1)
            proj_mm(stp[0][:, 0:512], wq_sb, 0, xts)
            nc.vector.tensor_scalar_add(
                qt[0][:, tb * 512:(tb + 1) * 512], stp[0][:, 0:512],
                bqc_sb[:, 0:1],
            )
            # psq(p1) -> stp0[:, 512:1024]
            proj_mm(stp[0][:, 512:1024], wq_sb, 1, xts)
            nc.vector.tensor_scalar_add(
                qt[1][:, tb * 512:(tb + 1) * 512], stp[0][:, 512:1024],
                bqc_sb[:, 1:2],
            )
            if tb > 0:
                norm_rb(0, stp[0][:, 0:512])
                norm_part2(0, stp[0][:, 0:512])
                norm_muls(0, tb - 1)
            # psk(p0) -> stp1[:, 0:512]
            proj_mm(stp[1][:, 0:512], wk_sb, 0, xts)
            nc.vector.tensor_scalar_add(
                kta[0][0:64, tb * 512:(tb + 1) * 512], stp[1][0:64, 0:512],
                bkc_sb[0:64, 0:1],
            )
            nc.vector.tensor_scalar_add(
                ktb[0][64:128, tb * 512:(tb + 1) * 512], stp[1][64:128, 0:512],
                bkc_sb[64:128, 0:1],
            )
            if tb > 0:
                norm_rb(1, stp[0][:, 512:1024])
                norm_part2(1, stp[0][:, 512:1024])
                norm_muls(1, tb - 1)
            # psk(p1) -> stp1[:, 512:1024]
            proj_mm(stp[1][:, 512:1024], wk_sb, 1, xts)
            nc.vector.tensor_scalar_add(
                kta[1][0:64, tb * 512:(tb + 1) * 512], stp[1][0:64, 512:1024],
                bkc_sb[0:64, 1:2],
            )
            nc.vector.tensor_scalar_add(
                ktb[1][64:128, tb * 512:(tb + 1) * 512], stp[1][64:128, 512:1024],
                bkc_sb[64:128, 1:2],
            )
            # psv: 4 sub-blocks of 128 t-rows -> ua0/ub0/ua1/ub1 [:, 0:256]
            psv_slots = [ua[0], ub[0], ua[1], ub[1]]
            for sub in range(4):
                j = tb * 4 + sub
                psv_t = psv_slots[sub]
                for db in range(8):
                    nc.tensor.matmul(
                        psv_t[:, 0:256],
                        xts[db][:, sub * 128:(sub + 1) * 128],
                        wv_sb[:, db * 256:(db + 1) * 256],
                        start=(db == 0), stop=(db == 7),
                    )
                # scatter v (+bias): v0|v1 -> [64:192], v2|v3 -> [256:384]
                nc.vector.tensor_add(
                    vt[:, j * 384 + 64: j * 384 + 192],
                    psv_t[:, 0:128], bvb_sb[:, 0:128],
                )
                nc.vector.tensor_add(
                    vt[:, j * 384 + 256: j * 384 + 384],
                    psv_t[:, 128:256], bvb_sb[:, 128:256],
                )

            # ---------- output projection for tb-1 ----------
            if tb > 0:
                oproj(tb - 1)

            # ---------- SDPA for i = tb ----------
            i = tb
            jmax = 4 * i + 4
            ets = {}
            for j in range(jmax):
                t = j - 4 * i
                cs = 128 * t if t >= 0 else 0
                w = 512 - cs
                for p in range(2):
                    nc.tensor.matmul(
                        stp[p][:, cs:512],
                        kta[p][:, j * 128:(j + 1) * 128],
                        qt[p][:, i * 512 + cs:(i + 1) * 512],
                        start=True, stop=True,
                    )
                    nc.tensor.matmul(
                        stp[p][:, 512 + cs:1024],
                        ktb[p][:, j * 128:(j + 1) * 128],
                        qt[p][:, i * 512 + cs:(i + 1) * 512],
                        start=True, stop=True,
                    )
                    et = epool.tile([128, 1024], BF16, name="et", tag="et")
                    src = stp[p][:].rearrange("p (h w) -> p h w", h=2)[:, :, cs:512]
                    dst = et[:].rearrange("p (h w) -> p h w", h=2)[:, :, cs:512]
                    nc.scalar.activation(dst, src, EXP, scale=0.125)
                    if t >= 0:
                        nc.vector.tensor_mul(dst, dst, mtri3[:, :, 0:w])
                    ets[(j, p)] = et
                if j >= 2:
                    _pv(nc, vt, ua, ub, ets, j - 2, jmax, 4 * i)
                    del ets[(j - 2, 0)], ets[(j - 2, 1)]
            _pv(nc, vt, ua, ub, ets, jmax - 2, jmax, 4 * i)
            _pv(nc, vt, ua, ub, ets, jmax - 1, jmax, 4 * i)

        # tail: norm + oproj for last block
        tb1 = NT - 1
        norm_part1(tb1, tail=True)
        norm_rb(0, stp[0][:, 0:512])
        norm_part2(0, stp[0][:, 0:512])
        norm_muls(0, tb1)
        norm_rb(1, stp[0][:, 512:1024])
        norm_part2(1, stp[0][:, 512:1024])
        norm_muls(1, tb1)
        oproj(tb1, tail=True)

    nc.compile()
    return nc


def _pv(nc, vt, ua, ub, ets, j, jmax, i4):
    t = j - i4
    cs = 128 * t if t >= 0 else 0
    st_flags = dict(start=(j == 0), stop=(j == jmax - 1), skip_group_check=True)
    for p in range(2):
        et = ets[(j, p)]
        base = j * 384 + 192 * p
        nc.tensor.matmul(
            ua[p][:, cs:512], vt[:, base: base + 128], et[:, cs:512], **st_flags
        )
        nc.tensor.matmul(
            ub[p][:, cs:512], vt[:, base + 128: base + 256],
            et[:, 512 + cs:1024], **st_flags
        )


def _get_built(T):
    if T not in _BUILD_CACHE:
        _BUILD_CACHE[T] = _build(T)
    return _BUILD_CACHE[T]


def _rearr_w(w):  # [1024, 256] -> [128, 8*256] (d-block major free dim)
    return np.ascontiguousarray(
        w.reshape(8, 128, 256).transpose(1, 0, 2).reshape(128, 8 * 256)
    )


def _numpy_ref(x, mask, Wq, bq, Wk, bk, Wv, bv, Wo, bo):
    T = x.shape[1]
    q = (x @ Wq + bq).reshape(B, T, H, DK).transpose(0, 2, 1, 3)
    k = (x @ Wk + bk).reshape(B, T, H, DK).transpose(0, 2, 1, 3)
    v = (x @ Wv + bv).reshape(B, T, H, DK).transpose(0, 2, 1, 3)
    s = np.einsum("bhqd,bhkd->bhqk", q, k) / np.sqrt(np.float32(DK))
    s = np.where(mask, s, s - 1e9)
    s = s - s.max(axis=-1, keepdims=True)
    e = np.exp(s)
    p = e / e.sum(axis=-1, keepdims=True)
    o = np.einsum("bhqk,bhkd->bhqd", p, v).transpose(0, 2, 1, 3).reshape(B, T, D)
    return (o @ Wo + bo).astype(np.float32)


def kernel(x, mask, Wq, bq, Wk, bk, Wv, bv, Wo, bo):
    from concourse import bass_utils

    x = np.ascontiguousarray(np.asarray(x, dtype=np.float32))
    mask = np.asarray(mask)
    T = x.shape[1]

    causal = bool(
        np.array_equal(mask[0, 0], np.tril(np.ones((T, T), dtype=bool)))
    )
    if not causal or x.shape != (B, T, D) or T % 512 != 0:
        return _numpy_ref(
            np.asarray(x, np.float32), mask,
            np.asarray(Wq, np.float32), np.asarray(bq, np.float32),
            np.asarray(Wk, np.float32), np.asarray(bk, np.float32),
            np.asarray(Wv, np.float32), np.asarray(bv, np.float32),
            np.asarray(Wo, np.float32), np.asarray(bo, np.float32),
        )

    in_maps = _make_in_maps(dict(x=x, Wq=Wq, bq=bq, Wk=Wk, bk=bk,
                                 Wv=Wv, bv=bv, Wo=Wo))
    nc = _get_built(T)
    res = bass_utils.run_bass_kernel_spmd(nc, in_maps, core_ids=list(range(NCORES)))

    out = np.zeros((B, T, D), np.float32)
    for c in range(NCORES):
        out[c // 4] += res.results[c]["ot"].T
    out += np.asarray(bo, np.float32)
    return out


def _make_in_maps(inputs):
    import ml_dtypes

    x = np.ascontiguousarray(np.asarray(inputs["x"], np.float32))
    T = x.shape[1]
    NJ = T // 128
    Wq = np.asarray(inputs["Wq"], np.float32)
    Wk = np.asarray(inputs["Wk"], np.float32)
    Wv = np.asarray(inputs["Wv"], np.float32)
    Wo = np.asarray(inputs["Wo"], np.float32)
    bq = np.asarray(inputs["bq"], np.float32)
    bk = np.asarray(inputs["bk"], np.float32)
    bv = np.asarray(inputs["bv"], np.float32)

    e2 = np.zeros((128, 128), np.float32)
    e2[64, 0:64] = 1.0
    e2[0, 64:128] = 1.0
    vpad = np.zeros((128, NJ * 64), ml_dtypes.bfloat16)
    vpad[:, ::64] = 1.0
    mtri = np.zeros((128, 1024), ml_dtypes.bfloat16)
    tri = (np.arange(128)[:, None] <= np.arange(512)[None, :])
    mtri[:, 0:512] = tri
    mtri[:, 512:1024] = tri

    xts = [np.ascontiguousarray(x[b].T) for b in range(B)]

    in_maps = []
    for c in range(NCORES):
        b, g = divmod(c, 4)
        cols = slice(g * CW, (g + 1) * CW)
        rows = slice(g * CW, (g + 1) * CW)
        wo_g = Wo[rows]  # [256, 1024]
        in_maps.append({
            "xt": xts[b],
            "wq": _rearr_w(Wq[:, cols]),
            "wk": _rearr_w(Wk[:, cols]),
            "wv": _rearr_w(Wv[:, cols]),
            "wo": np.ascontiguousarray(
                wo_g.reshape(2, 128, 1024).transpose(1, 0, 2).reshape(128, 2048)
            ),
            "bqc": np.ascontiguousarray(bq[cols].reshape(2, 128).T),
            "bkc": np.ascontiguousarray(bk[cols].reshape(2, 128).T),
            "bvb": np.ascontiguousarray(
                np.broadcast_to(bv[cols][None, :], (128, 256)).copy()
            ),
            "e2sel": e2,
            "vpad": vpad,
            "mtri": mtri,
        })

    return in_maps


# revision 6
# speedup vs baseline: 1.0679x; 1.0248x over previous
"""Multi-head causal attention (B=2, T=4096, D=1024, H=16) on 8 trn2 cores.

Sharding: core c = 4*b + g handles batch b and head-group g (4 heads).
Merged single-pass pipeline per 512-col q-block tb:
  proj(tb) -> norm(tb-1) -> oproj(tb-1) -> SDPA(i=tb)
SDPA streams in bf16 (q/k/v/probs); Z rows fused into PV via [v|ones]
stationary tiles; causal mask via DVE multiply with a triangular bf16
constant. Host sums the per-core partial O^T and adds bo.
"""
import numpy as np

B, T0, D, H = 2, 4096, 1024, 16
DK = D // H          # 64
NCORES = 8
HPC = H // 4         # 4 heads per core
CW = HPC * DK        # 256 head-columns per core

_BUILD_CACHE = {}


def _build(T):
    import concourse.bacc as bacc
    import concourse.mybir as mybir
    import concourse.tile as tile
    from contextlib import ExitStack

    F32 = mybir.dt.float32
    F32R = mybir.dt.float32r
    BF16 = mybir.dt.bfloat16
    EXP = mybir.ActivationFunctionType.Exp

    NT = T // 512    # q-blocks of 512
    NJ = T // 128    # k-blocks of 128
    VTW = NJ * 384 + 64   # per j: ones|v0|v1|ones2|v2|v3, plus final ones

    nc = bacc.Bacc("TRN2", target_bir_lowering=False, debug=False, num_devices=8)

    xt_d = nc.dram_tensor("xt", [D, T], F32R, kind="ExternalInput")
    wq_d = nc.dram_tensor("wq", [128, 8 * 256], F32R, kind="ExternalInput")
    wk_d = nc.dram_tensor("wk", [128, 8 * 256], F32R, kind="ExternalInput")
    wv_d = nc.dram_tensor("wv", [128, 8 * 256], F32R, kind="ExternalInput")
    wo_d = nc.dram_tensor("wo", [128, 2 * 1024], BF16, kind="ExternalInput")
    bqc_d = nc.dram_tensor("bqc", [128, 2], F32, kind="ExternalInput")
    bkc_d = nc.dram_tensor("bkc", [128, 2], F32, kind="ExternalInput")
    bvb_d = nc.dram_tensor("bvb", [128, 256], F32, kind="ExternalInput")
    e2_d = nc.dram_tensor("e2sel", [128, 128], F32R, kind="ExternalInput")
    vpad_d = nc.dram_tensor("vpad", [128, NJ * 64], BF16, kind="ExternalInput")
    mtri_d = nc.dram_tensor("mtri", [128, 1024], BF16, kind="ExternalInput")
    ot_d = nc.dram_tensor("ot", [D, T], F32, kind="ExternalOutput")

    with tile.TileContext(nc) as tc, ExitStack() as ctx:
        ctx.enter_context(nc.allow_low_precision(reason="bf16/fp32r by design"))

        # ---- persistent SBUF ----
        per = ctx.enter_context(tc.tile_pool(name="persist", bufs=1))
        qt = [per.tile([128, T], BF16, name=f"qt{p}", tag=f"qt{p}") for p in range(2)]
        kta = [per.tile([128, T], BF16, name=f"kta{p}", tag=f"kta{p}") for p in range(2)]
        ktb = [per.tile([128, T], BF16, name=f"ktb{p}", tag=f"ktb{p}") for p in range(2)]
        vt = per.tile([128, VTW], BF16, name="vt", tag="vt")
        wq_sb = per.tile([128, 2048], F32R, name="wq", tag="wq")
        wk_sb = per.tile([128, 2048], F32R, name="wk", tag="wk")
        wv_sb = per.tile([128, 2048], F32R, name="wv", tag="wv")
        wo_sb = per.tile([128, 2048], BF16, name="wo", tag="wo")
        e2_sb = per.tile([128, 128], F32R, name="e2", tag="e2")
        mtri_sb = per.tile([128, 1024], BF16, name="mtri", tag="mtri")
        bqc_sb = per.tile([128, 2], F32, name="bqc", tag="bqc")
        bkc_sb = per.tile([128, 2], F32, name="bkc", tag="bkc")
        bvb_sb = per.tile([128, 256], F32, name="bvb", tag="bvb")
        cx = [per.tile([128, T], BF16, name=f"cx{p}", tag=f"cx{p}") for p in range(2)]
        zr = [per.tile([128, 512], F32R, name=f"zr{p}", tag=f"zr{p}") for p in range(2)]
        rr = [per.tile([128, 512], F32, name=f"rr{p}", tag=f"rr{p}") for p in range(2)]

        # ---- persistent PSUM (8 banks, region-aliased across phases) ----
        ps = ctx.enter_context(tc.tile_pool(name="ps", bufs=1, space="PSUM"))
        stp = [ps.tile([128, 1024], F32, name=f"stp{p}", tag=f"stp{p}")
               for p in range(2)]
        ua = [ps.tile([128, 512], F32, name=f"ua{p}", tag=f"ua{p}") for p in range(2)]
        ub = [ps.tile([128, 512], F32, name=f"ub{p}", tag=f"ub{p}") for p in range(2)]

        # ---- initial DMAs, ordered so proj(0) can start ASAP ----
        xpool = ctx.enter_context(tc.tile_pool(name="xts", bufs=12))

        def load_x(tb, wq_interleave=False):
            xts = []
            for db in range(8):
                if wq_interleave:
                    nc.sync.dma_start(
                        wq_sb[:, db * 256:(db + 1) * 256],
                        wq_d.ap()[:, db * 256:(db + 1) * 256],
                    )
                xtile = xpool.tile([128, 512], F32R, name="xt", tag="xt")
                nc.sync.dma_start(
                    xtile[:],
                    xt_d.ap()[db * 128:(db + 1) * 128, tb * 512:(tb + 1) * 512],
                )
                xts.append(xtile)
            return xts

        xts_cur = load_x(0, wq_interleave=True)
        nc.sync.dma_start(wk_sb[:], wk_d.ap()[:])
        nc.sync.dma_start(wv_sb[:], wv_d.ap()[:])
        nc.sync.dma_start(bqc_sb[:], bqc_d.ap()[:])
        nc.sync.dma_start(bkc_sb[:], bkc_d.ap()[:])
        nc.sync.dma_start(bvb_sb[:], bvb_d.ap()[:])
        nc.sync.dma_start(e2_sb[:], e2_d.ap()[:])
        nc.sync.dma_start(mtri_sb[:], mtri_d.ap()[:])
        # ones/zero pad columns of the v-tiles (col 64 and 256 of each j blk)
        vt3 = vt[:, 0:NJ * 384].rearrange("p (j c) -> p j c", c=384)
        vsrc = vpad_d.ap()[:].rearrange("p (j c) -> p j c", c=64)
        nc.sync.dma_start(vt3[:, :, 0:64], vsrc)
        nc.sync.dma_start(vt3[:, :, 192:256], vsrc)
        nc.sync.dma_start(vt[:, NJ * 384: NJ * 384 + 64],
                          vpad_d.ap()[:, 0:64])
        nc.sync.dma_start(wo_sb[:], wo_d.ap()[:])

        opool = ctx.enter_context(tc.tile_pool(name="otile", bufs=6))
        epool = ctx.enter_context(tc.tile_pool(name="expt", bufs=14))

        mtri3 = mtri_sb[:].rearrange("p (h w) -> p h w", h=2)

        def proj_mm(out_ps, w_sb, p, xts):
            for db in range(8):
                nc.tensor.matmul(
                    out_ps,
                    w_sb[:, db * 256 + p * 128: db * 256 + (p + 1) * 128],
                    xts[db][:],
                    start=(db == 0), stop=(db == 7),
                )

        def norm_part1(tb1, tail=False):
            # Z rows out of PSUM: Z_a -> zr row 0, Z_b -> zr row 64
            # (at the tail ACT is idle -> use it for the PSUM reads)
            eng = nc.scalar.copy if tail else nc.vector.tensor_copy
            for p in range(2):
                eng(zr[p][0:64, :], ua[p][0:64, :])
                eng(zr[p][64:128, :], ub[p][64:128, :])

        def norm_rb(p, region):
            # e2 matmul: rb rows 0:64 <- Z_b (zr row 64), rows 64:128 <- Z_a
            nc.tensor.matmul(region, e2_sb[:], zr[p][:], start=True, stop=True)

        def norm_part2(p, region):
            nc.vector.reciprocal_approx_fast(out=rr[p][:], in_=region)

        def norm_muls(p, tb1):
            nc.vector.tensor_mul(
                cx[p][0:64, tb1 * 512:(tb1 + 1) * 512],
                ua[p][64:128, :], rr[p][64:128, :],
            )
            nc.vector.tensor_mul(
                cx[p][64:128, tb1 * 512:(tb1 + 1) * 512],
                ub[p][0:64, :], rr[p][0:64, :],
            )

        def oproj_ob(tb1, ob, tail=False):
            slots = [ua[0], ub[0], ua[1], ub[1]] if tail else [ua[0], ub[0]]
            po = slots[ob % len(slots)][:]
            nc.tensor.matmul(
                po,
                wo_sb[:, ob * 128:(ob + 1) * 128],
                cx[0][:, tb1 * 512:(tb1 + 1) * 512],
                start=True, stop=False, skip_group_check=True,
            )
            nc.tensor.matmul(
                po,
                wo_sb[:, 1024 + ob * 128: 1024 + (ob + 1) * 128],
                cx[1][:, tb1 * 512:(tb1 + 1) * 512],
                start=False, stop=True, skip_group_check=True,
            )
            ot_t = opool.tile([128, 512], F32, name="ot", tag="ot")
            (nc.scalar.copy if tail else nc.vector.tensor_copy)(ot_t[:], po)
            nc.sync.dma_start(
                ot_d.ap()[ob * 128:(ob + 1) * 128, tb1 * 512:(tb1 + 1) * 512],
                ot_t[:],
            )

        def oproj(tb1, tail=False):
            for ob in range(8):
                oproj_ob(tb1, ob, tail=tail)

        for tb in range(NT):
            xts = xts_cur
            if tb + 1 < NT:
                xts_cur = load_x(tb + 1)

            # ---------- projections for tb (+ norm(tb-1) interleaved) ----------
            # psq(p0) -> stp0[:, 0:512]
            if tb > 0:
                norm_part1(tb - 1)
            proj_mm(stp[0][:, 0:512], wq_sb, 0, xts)
            nc.vector.tensor_scalar_add(
                qt[0][:, tb * 512:(tb + 1) * 512], stp[0][:, 0:512],
                bqc_sb[:, 0:1],
            )
            # psq(p1) -> stp0[:, 512:1024]
            proj_mm(stp[0][:, 512:1024], wq_sb, 1, xts)
            nc.vector.tensor_scalar_add(
                qt[1][:, tb * 512:(tb + 1) * 512], stp[0][:, 512:1024],
                bqc_sb[:, 1:2],
            )
            if tb > 0:
                norm_rb(0, stp[0][:, 0:512])
                norm_part2(0, stp[0][:, 0:512])
                norm_muls(0, tb - 1)
            # zero halves of this tb's kt slices (before SDPA(tb) STs)
            for p in range(2):
                nc.vector.memset(kta[p][64:128, tb * 512:(tb + 1) * 512], 0.0)
                nc.vector.memset(ktb[p][0:64, tb * 512:(tb + 1) * 512], 0.0)
            # psk(p0) -> stp1[:, 0:512]
            proj_mm(stp[1][:, 0:512], wk_sb, 0, xts)
            nc.vector.tensor_scalar_add(
                kta[0][0:64, tb * 512:(tb + 1) * 512], stp[1][0:64, 0:512],
                bkc_sb[0:64, 0:1],
            )
            nc.vector.tensor_scalar_add(
                ktb[0][64:128, tb * 512:(tb + 1) * 512], stp[1][64:128, 0:512],
                bkc_sb[64:128, 0:1],
            )
            if tb > 0:
                norm_rb(1, stp[0][:, 512:1024])
                norm_part2(1, stp[0][:, 512:1024])
                norm_muls(1, tb - 1)
            # psk(p1) -> stp1[:, 512:1024]
            proj_mm(stp[1][:, 512:1024], wk_sb, 1, xts)
            nc.vector.tensor_scalar_add(
                kta[1][0:64, tb * 512:(tb + 1) * 512], stp[1][0:64, 512:1024],
                bkc_sb[0:64, 1:2],
            )
            nc.vector.tensor_scalar_add(
                ktb[1][64:128, tb * 512:(tb + 1) * 512], stp[1][64:128, 512:1024],
                bkc_sb[64:128, 1:2],
            )
            # psv: 4 sub-blocks of 128 t-rows -> ua0/ub0/ua1/ub1 [:, 0:256]
            psv_slots = [ua[0], ub[0], ua[1], ub[1]]
            for sub in range(4):
                j = tb * 4 + sub
                psv_t = psv_slots[sub]
                for db in range(8):
                    nc.tensor.matmul(
                        psv_t[:, 0:256],
                        xts[db][:, sub * 128:(sub + 1) * 128],
                        wv_sb[:, db * 256:(db + 1) * 256],
                        start=(db == 0), stop=(db == 7),
                    )
                # scatter v (+bias): v0|v1 -> [64:192], v2|v3 -> [256:384]
                nc.vector.tensor_add(
                    vt[:, j * 384 + 64: j * 384 + 192],
                    psv_t[:, 0:128], bvb_sb[:, 0:128],
                )
                nc.vector.tensor_add(
                    vt[:, j * 384 + 256: j * 384 + 384],
                    psv_t[:, 128:256], bvb_sb[:, 128:256],
                )

            # ---------- SDPA for i = tb; pre-phase STs overlap oproj ----------
            i = tb
            jmax = 4 * i + 4
            ets = {}

            def emit_st_exp(j):
                t = j - 4 * i
                cs = 128 * t if t >= 0 else 0
                w = 512 - cs
                for p in range(2):
                    nc.tensor.matmul(
                        stp[p][:, cs:512],
                        kta[p][:, j * 128:(j + 1) * 128],
                        qt[p][:, i * 512 + cs:(i + 1) * 512],
                        start=True, stop=True,
                    )
                    nc.tensor.matmul(
                        stp[p][:, 512 + cs:1024],
                        ktb[p][:, j * 128:(j + 1) * 128],
                        qt[p][:, i * 512 + cs:(i + 1) * 512],
                        start=True, stop=True,
                    )
                    et = epool.tile([128, 1024], BF16, name="et", tag="et")
                    esrc = stp[p][:].rearrange("p (h w) -> p h w", h=2)[:, :, cs:512]
                    dst = et[:].rearrange("p (h w) -> p h w", h=2)[:, :, cs:512]
                    nc.scalar.activation(dst, esrc, EXP, scale=0.125)
                    if t >= 0:
                        nc.vector.tensor_mul(dst, dst, mtri3[:, :, 0:w])
                    ets[(j, p)] = et

            P = min(5, jmax)
            obq = list(range(8)) if tb > 0 else []
            for j in range(P):
                emit_st_exp(j)
                if j >= 1:
                    for _ in range(2):
                        if obq:
                            oproj_ob(tb - 1, obq.pop(0))
            while obq:
                oproj_ob(tb - 1, obq.pop(0))
            for j in range(P, jmax):
                emit_st_exp(j)
                _pv(nc, vt, ua, ub, ets, j - P, jmax, 4 * i)
                del ets[(j - P, 0)], ets[(j - P, 1)]
            for jt in range(jmax - P, jmax):
                _pv(nc, vt, ua, ub, ets, jt, jmax, 4 * i)

        # tail: norm + oproj for last block
        tb1 = NT - 1
        norm_part1(tb1, tail=True)
        norm_rb(0, stp[0][:, 0:512])
        norm_part2(0, stp[0][:, 0:512])
        norm_muls(0, tb1)
        norm_rb(1, stp[0][:, 512:1024])
        norm_part2(1, stp[0][:, 512:1024])
        norm_muls(1, tb1)
        oproj(tb1, tail=True)

    nc.compile()
    return nc


def _pv(nc, vt, ua, ub, ets, j, jmax, i4):
    t = j - i4
    cs = 128 * t if t >= 0 else 0
    st_flags = dict(start=(j == 0), stop=(j == jmax - 1), skip_group_check=True)
    for p in range(2):
        et = ets[(j, p)]
        base = j * 384 + 192 * p
        nc.tensor.matmul(
            ua[p][:, cs:512], vt[:, base: base + 128], et[:, cs:512], **st_flags
        )
        nc.tensor.matmul(
            ub[p][:, cs:512], vt[:, base + 128: base + 256],
            et[:, 512 + cs:1024], **st_flags
        )


def _get_built(T):
    if T not in _BUILD_CACHE:
        _BUILD_CACHE[T] = _build(T)
    return _BUILD_CACHE[T]


def _rearr_w(w):  # [1024, 256] -> [128, 8*256] (d-block major free dim)
    return np.ascontiguousarray(
        w.reshape(8, 128, 256).transpose(1, 0, 2).reshape(128, 8 * 256)
    )


def _numpy_ref(x, mask, Wq, bq, Wk, bk, Wv, bv, Wo, bo):
    T = x.shape[1]
    q = (x @ Wq + bq).reshape(B, T, H, DK).transpose(0, 2, 1, 3)
    k = (x @ Wk + bk).reshape(B, T, H, DK).transpose(0, 2, 1, 3)
    v = (x @ Wv + bv).reshape(B, T, H, DK).transpose(0, 2, 1, 3)
    s = np.einsum("bhqd,bhkd->bhqk", q, k) / np.sqrt(np.float32(DK))
    s = np.where(mask, s, s - 1e9)
    s = s - s.max(axis=-1, keepdims=True)
    e = np.exp(s)
    p = e / e.sum(axis=-1, keepdims=True)
    o = np.einsum("bhqk,bhkd->bhqd", p, v).transpose(0, 2, 1, 3).reshape(B, T, D)
    return (o @ Wo + bo).astype(np.float32)


def kernel(x, mask, Wq, bq, Wk, bk, Wv, bv, Wo, bo):
    from concourse import bass_utils

    x = np.ascontiguousarray(np.asarray(x, dtype=np.float32))
    mask = np.asarray(mask)
    T = x.shape[1]

    causal = bool(
        np.array_equal(mask[0, 0], np.tril(np.ones((T, T), dtype=bool)))
    )
    if not causal or x.shape != (B, T, D) or T % 512 != 0:
        return _numpy_ref(
            np.asarray(x, np.float32), mask,
            np.asarray(Wq, np.float32), np.asarray(bq, np.float32),
            np.asarray(Wk, np.float32), np.asarray(bk, np.float32),
            np.asarray(Wv, np.float32), np.asarray(bv, np.float32),
            np.asarray(Wo, np.float32), np.asarray(bo, np.float32),
        )

    in_maps = _make_in_maps(dict(x=x, Wq=Wq, bq=bq, Wk=Wk, bk=bk,
                                 Wv=Wv, bv=bv, Wo=Wo))
    nc = _get_built(T)
    res = bass_utils.run_bass_kernel_spmd(nc, in_maps, core_ids=list(range(NCORES)))

    out = np.zeros((B, T, D), np.float32)
    for c in range(NCORES):
        out[c // 4] += res.results[c]["ot"].T
    out += np.asarray(bo, np.float32)
    return out


def _make_in_maps(inputs):
    import ml_dtypes

    x = np.ascontiguousarray(np.asarray(inputs["x"], np.float32))
    T = x.shape[1]
    NJ = T // 128
    Wq = np.asarray(inputs["Wq"], np.float32)
    Wk = np.asarray(inputs["Wk"], np.float32)
    Wv = np.asarray(inputs["Wv"], np.float32)
    Wo = np.asarray(inputs["Wo"], np.float32)
    bq = np.asarray(inputs["bq"], np.float32)
    bk = np.asarray(inputs["bk"], np.float32)
    bv = np.asarray(inputs["bv"], np.float32)

    e2 = np.zeros((128, 128), np.float32)
    e2[64, 0:64] = 1.0
    e2[0, 64:128] = 1.0
    vpad = np.zeros((128, NJ * 64), ml_dtypes.bfloat16)
    vpad[:, ::64] = 1.0
    mtri = np.zeros((128, 1024), ml_dtypes.bfloat16)
    tri = (np.arange(128)[:, None] <= np.arange(512)[None, :])
    mtri[:, 0:512] = tri
    mtri[:, 512:1024] = tri

    xts = [np.ascontiguousarray(x[b].T) for b in range(B)]

    in_maps = []
    for c in range(NCORES):
        b, g = divmod(c, 4)
        cols = slice(g * CW, (g + 1) * CW)
        rows = slice(g * CW, (g + 1) * CW)
        wo_g = Wo[rows]  # [256, 1024]
        in_maps.append({
            "xt": xts[b],
            "wq": _rearr_w(Wq[:, cols]),
            "wk": _rearr_w(Wk[:, cols]),
            "wv": _rearr_w(Wv[:, cols]),
            "wo": np.ascontiguousarray(
                wo_g.reshape(2, 128, 1024).transpose(1, 0, 2).reshape(128, 2048)
            ).astype(ml_dtypes.bfloat16),
            "bqc": np.ascontiguousarray(bq[cols].reshape(2, 128).T),
            "bkc": np.ascontiguousarray(bk[cols].reshape(2, 128).T),
            "bvb": np.ascontiguousarray(
                np.broadcast_to(bv[cols][None, :], (128, 256)).copy()
            ),
            "e2sel": e2,
            "vpad": vpad,
            "mtri": mtri,
        })

    return in_maps


# revision 7
# speedup vs baseline: 1.0694x; 1.0014x over previous
"""Multi-head causal attention (B=2, T=4096, D=1024, H=16) on 8 trn2 cores.

Sharding: core c = 4*b + g handles batch b and head-group g (4 heads).
Merged single-pass pipeline per 512-col q-block tb:
  proj(tb) -> norm(tb-1) -> oproj(tb-1) -> SDPA(i=tb)
SDPA streams in bf16 (q/k/v/probs); Z rows fused into PV via [v|ones]
stationary tiles; causal mask via DVE multiply with a triangular bf16
constant. Host sums the per-core partial O^T and adds bo.
"""
import numpy as np

B, T0, D, H = 2, 4096, 1024, 16
DK = D // H          # 64
NCORES = 8
HPC = H // 4         # 4 heads per core
CW = HPC * DK        # 256 head-columns per core

_BUILD_CACHE = {}


def _build(T):
    import concourse.bacc as bacc
    import concourse.mybir as mybir
    import concourse.tile as tile
    from contextlib import ExitStack

    F32 = mybir.dt.float32
    F32R = mybir.dt.float32r
    BF16 = mybir.dt.bfloat16
    EXP = mybir.ActivationFunctionType.Exp

    NT = T // 512    # q-blocks of 512
    NJ = T // 128    # k-blocks of 128
    VTW = NJ * 384 + 64   # per j: ones|v0|v1|ones2|v2|v3, plus final ones

    nc = bacc.Bacc("TRN2", target_bir_lowering=False, debug=False, num_devices=8)

    xt_d = nc.dram_tensor("xt", [D, T], F32R, kind="ExternalInput")
    wq_d = nc.dram_tensor("wq", [128, 8 * 256], F32R, kind="ExternalInput")
    wk_d = nc.dram_tensor("wk", [128, 8 * 256], F32R, kind="ExternalInput")
    wv_d = nc.dram_tensor("wv", [128, 8 * 256], F32R, kind="ExternalInput")
    wo_d = nc.dram_tensor("wo", [128, 2 * 1024], BF16, kind="ExternalInput")
    bqc_d = nc.dram_tensor("bqc", [128, 2], F32, kind="ExternalInput")
    bkc_d = nc.dram_tensor("bkc", [128, 2], F32, kind="ExternalInput")
    bvb_d = nc.dram_tensor("bvb", [128, 256], F32, kind="ExternalInput")
    e2_d = nc.dram_tensor("e2sel", [128, 128], F32R, kind="ExternalInput")
    vpad_d = nc.dram_tensor("vpad", [128, NJ * 64], BF16, kind="ExternalInput")
    mtri_d = nc.dram_tensor("mtri", [128, 1024], BF16, kind="ExternalInput")
    ot_d = nc.dram_tensor("ot", [D, T], BF16, kind="ExternalOutput")

    with tile.TileContext(nc) as tc, ExitStack() as ctx:
        ctx.enter_context(nc.allow_low_precision(reason="bf16/fp32r by design"))

        # ---- persistent SBUF ----
        per = ctx.enter_context(tc.tile_pool(name="persist", bufs=1))
        qt = [per.tile([128, T], BF16, name=f"qt{p}", tag=f"qt{p}") for p in range(2)]
        kta = [per.tile([128, T], BF16, name=f"kta{p}", tag=f"kta{p}") for p in range(2)]
        ktb = [per.tile([128, T], BF16, name=f"ktb{p}", tag=f"ktb{p}") for p in range(2)]
        vt = per.tile([128, VTW], BF16, name="vt", tag="vt")
        wq_sb = per.tile([128, 2048], F32R, name="wq", tag="wq")
        wk_sb = per.tile([128, 2048], F32R, name="wk", tag="wk")
        wv_sb = per.tile([128, 2048], F32R, name="wv", tag="wv")
        wo_sb = per.tile([128, 2048], BF16, name="wo", tag="wo")
        e2_sb = per.tile([128, 128], F32R, name="e2", tag="e2")
        mtri_sb = per.tile([128, 1024], BF16, name="mtri", tag="mtri")
        bqc_sb = per.tile([128, 2], F32, name="bqc", tag="bqc")
        bkc_sb = per.tile([128, 2], F32, name="bkc", tag="bkc")
        bvb_sb = per.tile([128, 256], F32, name="bvb", tag="bvb")
        cx = [per.tile([128, T], BF16, name=f"cx{p}", tag=f"cx{p}") for p in range(2)]
        zr = [per.tile([128, 512], F32R, name=f"zr{p}", tag=f"zr{p}") for p in range(2)]
        rr = [per.tile([128, 512], F32, name=f"rr{p}", tag=f"rr{p}") for p in range(2)]

        # ---- persistent PSUM (8 banks, region-aliased across phases) ----
        ps = ctx.enter_context(tc.tile_pool(name="ps", bufs=1, space="PSUM"))
        stp = [ps.tile([128, 1024], F32, name=f"stp{p}", tag=f"stp{p}")
               for p in range(2)]
        ua = [ps.tile([128, 512], F32, name=f"ua{p}", tag=f"ua{p}") for p in range(2)]
        ub = [ps.tile([128, 512], F32, name=f"ub{p}", tag=f"ub{p}") for p in range(2)]

        # ---- initial DMAs, ordered so proj(0) can start ASAP ----
        xpool = ctx.enter_context(tc.tile_pool(name="xts", bufs=12))

        def load_x(tb, wq_interleave=False):
            xts = []
            for db in range(8):
                if wq_interleave:
                    nc.sync.dma_start(
                        wq_sb[:, db * 256:(db + 1) * 256],
                        wq_d.ap()[:, db * 256:(db + 1) * 256],
                    )
                xtile = xpool.tile([128, 512], F32R, name="xt", tag="xt")
                nc.sync.dma_start(
                    xtile[:],
                    xt_d.ap()[db * 128:(db + 1) * 128, tb * 512:(tb + 1) * 512],
                )
                xts.append(xtile)
            return xts

        xts_cur = load_x(0, wq_interleave=True)
        nc.sync.dma_start(wk_sb[:], wk_d.ap()[:])
        nc.sync.dma_start(wv_sb[:], wv_d.ap()[:])
        nc.sync.dma_start(bqc_sb[:], bqc_d.ap()[:])
        nc.sync.dma_start(bkc_sb[:], bkc_d.ap()[:])
        nc.sync.dma_start(bvb_sb[:], bvb_d.ap()[:])
        nc.sync.dma_start(e2_sb[:], e2_d.ap()[:])
        nc.sync.dma_start(mtri_sb[:], mtri_d.ap()[:])
        # ones/zero pad columns of the v-tiles (col 64 and 256 of each j blk)
        vt3 = vt[:, 0:NJ * 384].rearrange("p (j c) -> p j c", c=384)
        vsrc = vpad_d.ap()[:].rearrange("p (j c) -> p j c", c=64)
        nc.sync.dma_start(vt3[:, :, 0:64], vsrc)
        nc.sync.dma_start(vt3[:, :, 192:256], vsrc)
        nc.sync.dma_start(vt[:, NJ * 384: NJ * 384 + 64],
                          vpad_d.ap()[:, 0:64])
        nc.sync.dma_start(wo_sb[:], wo_d.ap()[:])

        opool = ctx.enter_context(tc.tile_pool(name="otile", bufs=6))
        epool = ctx.enter_context(tc.tile_pool(name="expt", bufs=14))

        mtri3 = mtri_sb[:].rearrange("p (h w) -> p h w", h=2)

        def proj_mm(out_ps, w_sb, p, xts):
            for db in range(8):
                nc.tensor.matmul(
                    out_ps,
                    w_sb[:, db * 256 + p * 128: db * 256 + (p + 1) * 128],
                    xts[db][:],
                    start=(db == 0), stop=(db == 7),
                )

        def norm_part1(tb1, tail=False):
            # Z rows out of PSUM: Z_a -> zr row 0, Z_b -> zr row 64
            # (at the tail ACT is idle -> use it for the PSUM reads)
            eng = nc.scalar.copy if tail else nc.vector.tensor_copy
            for p in range(2):
                eng(zr[p][0:64, :], ua[p][0:64, :])
                eng(zr[p][64:128, :], ub[p][64:128, :])

        def norm_rb(p, region):
            # e2 matmul: rb rows 0:64 <- Z_b (zr row 64), rows 64:128 <- Z_a
            nc.tensor.matmul(region, e2_sb[:], zr[p][:], start=True, stop=True)

        def norm_part2(p, region):
            nc.vector.reciprocal_approx_fast(out=rr[p][:], in_=region)

        def norm_muls(p, tb1):
            nc.vector.tensor_mul(
                cx[p][0:64, tb1 * 512:(tb1 + 1) * 512],
                ua[p][64:128, :], rr[p][64:128, :],
            )
            nc.vector.tensor_mul(
                cx[p][64:128, tb1 * 512:(tb1 + 1) * 512],
                ub[p][0:64, :], rr[p][0:64, :],
            )

        def oproj_ob(tb1, ob, tail=False):
            slots = [ua[0], ub[0], ua[1], ub[1]] if tail else [ua[0], ub[0]]
            po = slots[ob % len(slots)][:]
            nc.tensor.matmul(
                po,
                wo_sb[:, ob * 128:(ob + 1) * 128],
                cx[0][:, tb1 * 512:(tb1 + 1) * 512],
                start=True, stop=False, skip_group_check=True,
            )
            nc.tensor.matmul(
                po,
                wo_sb[:, 1024 + ob * 128: 1024 + (ob + 1) * 128],
                cx[1][:, tb1 * 512:(tb1 + 1) * 512],
                start=False, stop=True, skip_group_check=True,
            )
            ot_t = opool.tile([128, 512], BF16, name="ot", tag="ot")
            (nc.scalar.copy if tail else nc.vector.tensor_copy)(ot_t[:], po)
            nc.sync.dma_start(
                ot_d.ap()[ob * 128:(ob + 1) * 128, tb1 * 512:(tb1 + 1) * 512],
                ot_t[:],
            )

        def oproj(tb1, tail=False):
            for ob in range(8):
                oproj_ob(tb1, ob, tail=tail)

        for tb in range(NT):
            xts = xts_cur
            if tb + 1 < NT:
                xts_cur = load_x(tb + 1)

            # ---------- projections for tb (+ norm(tb-1) interleaved) ----------
            # psq(p0) -> stp0[:, 0:512]
            if tb > 0:
                norm_part1(tb - 1)
            proj_mm(stp[0][:, 0:512], wq_sb, 0, xts)
            nc.vector.tensor_scalar_add(
                qt[0][:, tb * 512:(tb + 1) * 512], stp[0][:, 0:512],
                bqc_sb[:, 0:1],
            )
            # psq(p1) -> stp0[:, 512:1024]
            proj_mm(stp[0][:, 512:1024], wq_sb, 1, xts)
            nc.vector.tensor_scalar_add(
                qt[1][:, tb * 512:(tb + 1) * 512], stp[0][:, 512:1024],
                bqc_sb[:, 1:2],
            )
            if tb > 0:
                norm_rb(0, stp[0][:, 0:512])
                norm_part2(0, stp[0][:, 0:512])
                norm_muls(0, tb - 1)
            # zero halves of this tb's kt slices (before SDPA(tb) STs)
            for p in range(2):
                nc.vector.memset(kta[p][64:128, tb * 512:(tb + 1) * 512], 0.0)
                nc.vector.memset(ktb[p][0:64, tb * 512:(tb + 1) * 512], 0.0)
            # psk(p0) -> stp1[:, 0:512]
            proj_mm(stp[1][:, 0:512], wk_sb, 0, xts)
            nc.vector.tensor_scalar_add(
                kta[0][0:64, tb * 512:(tb + 1) * 512], stp[1][0:64, 0:512],
                bkc_sb[0:64, 0:1],
            )
            nc.vector.tensor_scalar_add(
                ktb[0][64:128, tb * 512:(tb + 1) * 512], stp[1][64:128, 0:512],
                bkc_sb[64:128, 0:1],
            )
            if tb > 0:
                norm_rb(1, stp[0][:, 512:1024])
                norm_part2(1, stp[0][:, 512:1024])
                norm_muls(1, tb - 1)
            # psk(p1) -> stp1[:, 512:1024]
            proj_mm(stp[1][:, 512:1024], wk_sb, 1, xts)
            nc.vector.tensor_scalar_add(
                kta[1][0:64, tb * 512:(tb + 1) * 512], stp[1][0:64, 512:1024],
                bkc_sb[0:64, 1:2],
            )
            nc.vector.tensor_scalar_add(
                ktb[1][64:128, tb * 512:(tb + 1) * 512], stp[1][64:128, 512:1024],
                bkc_sb[64:128, 1:2],
            )
            # psv: 4 sub-blocks of 128 t-rows -> ua0/ub0/ua1/ub1 [:, 0:256]
            psv_slots = [ua[0], ub[0], ua[1], ub[1]]
            for sub in range(4):
                j = tb * 4 + sub
                psv_t = psv_slots[sub]
                for db in range(8):
                    nc.tensor.matmul(
                        psv_t[:, 0:256],
                        xts[db][:, sub * 128:(sub + 1) * 128],
                        wv_sb[:, db * 256:(db + 1) * 256],
                        start=(db == 0), stop=(db == 7),
                    )
                # scatter v (+bias): v0|v1 -> [64:192], v2|v3 -> [256:384]
                nc.vector.tensor_add(
                    vt[:, j * 384 + 64: j * 384 + 192],
                    psv_t[:, 0:128], bvb_sb[:, 0:128],
                )
                nc.vector.tensor_add(
                    vt[:, j * 384 + 256: j * 384 + 384],
                    psv_t[:, 128:256], bvb_sb[:, 128:256],
                )

            # ---------- SDPA for i = tb; pre-phase STs overlap oproj ----------
            i = tb
            jmax = 4 * i + 4
            ets = {}

            def emit_st_exp(j):
                t = j - 4 * i
                cs = 128 * t if t >= 0 else 0
                w = 512 - cs
                for p in range(2):
                    nc.tensor.matmul(
                        stp[p][:, cs:512],
                        kta[p][:, j * 128:(j + 1) * 128],
                        qt[p][:, i * 512 + cs:(i + 1) * 512],
                        start=True, stop=True,
                    )
                    nc.tensor.matmul(
                        stp[p][:, 512 + cs:1024],
                        ktb[p][:, j * 128:(j + 1) * 128],
                        qt[p][:, i * 512 + cs:(i + 1) * 512],
                        start=True, stop=True,
                    )
                    et = epool.tile([128, 1024], BF16, name="et", tag="et")
                    esrc = stp[p][:].rearrange("p (h w) -> p h w", h=2)[:, :, cs:512]
                    dst = et[:].rearrange("p (h w) -> p h w", h=2)[:, :, cs:512]
                    nc.scalar.activation(dst, esrc, EXP, scale=0.125)
                    if t >= 0:
                        nc.vector.tensor_mul(dst, dst, mtri3[:, :, 0:w])
                    ets[(j, p)] = et

            P = min(5, jmax)
            obq = list(range(8)) if tb > 0 else []
            for j in range(P):
                emit_st_exp(j)
                if j >= 1:
                    for _ in range(2):
                        if obq:
                            oproj_ob(tb - 1, obq.pop(0))
            while obq:
                oproj_ob(tb - 1, obq.pop(0))
            for j in range(P, jmax):
                emit_st_exp(j)
                _pv(nc, vt, ua, ub, ets, j - P, jmax, 4 * i)
                del ets[(j - P, 0)], ets[(j - P, 1)]
            for jt in range(jmax - P, jmax):
                _pv(nc, vt, ua, ub, ets, jt, jmax, 4 * i)

        # tail: norm + oproj for last block
        tb1 = NT - 1
        norm_part1(tb1, tail=True)
        norm_rb(0, stp[0][:, 0:512])
        norm_part2(0, stp[0][:, 0:512])
        norm_muls(0, tb1)
        norm_rb(1, stp[0][:, 512:1024])
        norm_part2(1, stp[0][:, 512:1024])
        norm_muls(1, tb1)
        oproj(tb1, tail=True)

    nc.compile()
    return nc


def _pv(nc, vt, ua, ub, ets, j, jmax, i4):
    t = j - i4
    cs = 128 * t if t >= 0 else 0
    st_flags = dict(start=(j == 0), stop=(j == jmax - 1), skip_group_check=True)
    for p in range(2):
        et = ets[(j, p)]
        base = j * 384 + 192 * p
        nc.tensor.matmul(
            ua[p][:, cs:512], vt[:, base: base + 128], et[:, cs:512], **st_flags
        )
        nc.tensor.matmul(
            ub[p][:, cs:512], vt[:, base + 128: base + 256],
            et[:, 512 + cs:1024], **st_flags
        )


def _get_built(T):
    if T not in _BUILD_CACHE:
        _BUILD_CACHE[T] = _build(T)
    return _BUILD_CACHE[T]


def _rearr_w(w):  # [1024, 256] -> [128, 8*256] (d-block major free dim)
    return np.ascontiguousarray(
        w.reshape(8, 128, 256).transpose(1, 0, 2).reshape(128, 8 * 256)
    )


def _numpy_ref(x, mask, Wq, bq, Wk, bk, Wv, bv, Wo, bo):
    T = x.shape[1]
    q = (x @ Wq + bq).reshape(B, T, H, DK).transpose(0, 2, 1, 3)
    k = (x @ Wk + bk).reshape(B, T, H, DK).transpose(0, 2, 1, 3)
    v = (x @ Wv + bv).reshape(B, T, H, DK).transpose(0, 2, 1, 3)
    s = np.einsum("bhqd,bhkd->bhqk", q, k) / np.sqrt(np.float32(DK))
    s = np.where(mask, s, s - 1e9)
    s = s - s.max(axis=-1, keepdims=True)
    e = np.exp(s)
    p = e / e.sum(axis=-1, keepdims=True)
    o = np.einsum("bhqk,bhkd->bhqd", p, v).transpose(0, 2, 1, 3).reshape(B, T, D)
    return (o @ Wo + bo).astype(np.float32)


def kernel(x, mask, Wq, bq, Wk, bk, Wv, bv, Wo, bo):
    from concourse import bass_utils

    x = np.ascontiguousarray(np.asarray(x, dtype=np.float32))
    mask = np.asarray(mask)
    T = x.shape[1]

    causal = bool(
        np.array_equal(mask[0, 0], np.tril(np.ones((T, T), dtype=bool)))
    )
    if not causal or x.shape != (B, T, D) or T % 512 != 0:
        return _numpy_ref(
            np.asarray(x, np.float32), mask,
            np.asarray(Wq, np.float32), np.asarray(bq, np.float32),
            np.asarray(Wk, np.float32), np.asarray(bk, np.float32),
            np.asarray(Wv, np.float32), np.asarray(bv, np.float32),
            np.asarray(Wo, np.float32), np.asarray(bo, np.float32),
        )

    in_maps = _make_in_maps(dict(x=x, Wq=Wq, bq=bq, Wk=Wk, bk=bk,
                                 Wv=Wv, bv=bv, Wo=Wo))
    nc = _get_built(T)
    res = bass_utils.run_bass_kernel_spmd(nc, in_maps, core_ids=list(range(NCORES)))

    out = np.zeros((B, T, D), np.float32)
    for c in range(NCORES):
        out[c // 4] += res.results[c]["ot"].T.astype(np.float32)
    out += np.asarray(bo, np.float32)
    return out


def _make_in_maps(inputs):
    import ml_dtypes

    x = np.ascontiguousarray(np.asarray(inputs["x"], np.float32))
    T = x.shape[1]
    NJ = T // 128
    Wq = np.asarray(inputs["Wq"], np.float32)
    Wk = np.asarray(inputs["Wk"], np.float32)
    Wv = np.asarray(inputs["Wv"], np.float32)
    Wo = np.asarray(inputs["Wo"], np.float32)
    bq = np.asarray(inputs["bq"], np.float32)
    bk = np.asarray(inputs["bk"], np.float32)
    bv = np.asarray(inputs["bv"], np.float32)

    e2 = np.zeros((128, 128), np.float32)
    e2[64, 0:64] = 1.0
    e2[0, 64:128] = 1.0
    vpad = np.zeros((128, NJ * 64), ml_dtypes.bfloat16)
    vpad[:, ::64] = 1.0
    mtri = np.zeros((128, 1024), ml_dtypes.bfloat16)
    tri = (np.arange(128)[:, None] <= np.arange(512)[None, :])
    mtri[:, 0:512] = tri
    mtri[:, 512:1024] = tri

    xts = [np.ascontiguousarray(x[b].T) for b in range(B)]

    in_maps = []
    for c in range(NCORES):
        b, g = divmod(c, 4)
        cols = slice(g * CW, (g + 1) * CW)
        rows = slice(g * CW, (g + 1) * CW)
        wo_g = Wo[rows]  # [256, 1024]
        in_maps.append({
            "xt": xts[b],
            "wq": _rearr_w(Wq[:, cols]),
            "wk": _rearr_w(Wk[:, cols]),
            "wv": _rearr_w(Wv[:, cols]),
            "wo": np.ascontiguousarray(
                wo_g.reshape(2, 128, 1024).transpose(1, 0, 2).reshape(128, 2048)
            ).astype(ml_dtypes.bfloat16),
            "bqc": np.ascontiguousarray(bq[cols].reshape(2, 128).T),
            "bkc": np.ascontiguousarray(bk[cols].reshape(2, 128).T),
            "bvb": np.ascontiguousarray(
                np.broadcast_to(bv[cols][None, :], (128, 256)).copy()
            ),
            "e2sel": e2,
            "vpad": vpad,
            "mtri": mtri,
        })

    return in_maps


# revision 8
# speedup vs baseline: 1.0747x; 1.0050x over previous
"""Multi-head causal attention (B=2, T=4096, D=1024, H=16) on 8 trn2 cores.

Sharding: core c = 4*b + g handles batch b and head-group g (4 heads).
Merged single-pass pipeline per 512-col q-block tb:
  proj(tb) -> norm(tb-1) -> oproj(tb-1) -> SDPA(i=tb)
SDPA streams in bf16 (q/k/v/probs); Z rows fused into PV via [v|ones]
stationary tiles; causal mask via DVE multiply with a triangular bf16
constant. Host sums the per-core partial O^T and adds bo.
"""
import numpy as np

B, T0, D, H = 2, 4096, 1024, 16
DK = D // H          # 64
NCORES = 8
HPC = H // 4         # 4 heads per core
CW = HPC * DK        # 256 head-columns per core

_BUILD_CACHE = {}


def _build(T):
    import concourse.bacc as bacc
    import concourse.mybir as mybir
    import concourse.tile as tile
    from contextlib import ExitStack

    F32 = mybir.dt.float32
    F32R = mybir.dt.float32r
    BF16 = mybir.dt.bfloat16
    EXP = mybir.ActivationFunctionType.Exp

    NT = T // 512    # q-blocks of 512
    NJ = T // 128    # k-blocks of 128
    VTW = NJ * 384 + 64   # per j: ones|v0|v1|ones2|v2|v3, plus final ones

    nc = bacc.Bacc("TRN2", target_bir_lowering=False, debug=False, num_devices=8)

    xt_d = nc.dram_tensor("xt", [D, T], BF16, kind="ExternalInput")
    wq_d = nc.dram_tensor("wq", [128, 8 * 256], BF16, kind="ExternalInput")
    wk_d = nc.dram_tensor("wk", [128, 8 * 256], BF16, kind="ExternalInput")
    wv_d = nc.dram_tensor("wv", [128, 8 * 256], BF16, kind="ExternalInput")
    wo_d = nc.dram_tensor("wo", [128, 2 * 1024], BF16, kind="ExternalInput")
    bqc_d = nc.dram_tensor("bqc", [128, 2], F32, kind="ExternalInput")
    bkc_d = nc.dram_tensor("bkc", [128, 2], F32, kind="ExternalInput")
    bvb_d = nc.dram_tensor("bvb", [128, 256], F32, kind="ExternalInput")
    e2_d = nc.dram_tensor("e2sel", [128, 128], F32R, kind="ExternalInput")
    vpad_d = nc.dram_tensor("vpad", [128, NJ * 64], BF16, kind="ExternalInput")
    mtri_d = nc.dram_tensor("mtri", [128, 1024], BF16, kind="ExternalInput")
    ot_d = nc.dram_tensor("ot", [D, T], BF16, kind="ExternalOutput")

    with tile.TileContext(nc) as tc, ExitStack() as ctx:
        ctx.enter_context(nc.allow_low_precision(reason="bf16/fp32r by design"))

        # ---- persistent SBUF ----
        per = ctx.enter_context(tc.tile_pool(name="persist", bufs=1))
        qt = [per.tile([128, T], BF16, name=f"qt{p}", tag=f"qt{p}") for p in range(2)]
        kta = [per.tile([128, T], BF16, name=f"kta{p}", tag=f"kta{p}") for p in range(2)]
        ktb = [per.tile([128, T], BF16, name=f"ktb{p}", tag=f"ktb{p}") for p in range(2)]
        vt = per.tile([128, VTW], BF16, name="vt", tag="vt")
        wq_sb = per.tile([128, 2048], BF16, name="wq", tag="wq")
        wk_sb = per.tile([128, 2048], BF16, name="wk", tag="wk")
        wv_sb = per.tile([128, 2048], BF16, name="wv", tag="wv")
        wo_sb = per.tile([128, 2048], BF16, name="wo", tag="wo")
        e2_sb = per.tile([128, 128], F32R, name="e2", tag="e2")
        mtri_sb = per.tile([128, 1024], BF16, name="mtri", tag="mtri")
        bqc_sb = per.tile([128, 2], F32, name="bqc", tag="bqc")
        bkc_sb = per.tile([128, 2], F32, name="bkc", tag="bkc")
        bvb_sb = per.tile([128, 256], F32, name="bvb", tag="bvb")
        cx = [per.tile([128, T], BF16, name=f"cx{p}", tag=f"cx{p}") for p in range(2)]
        zr = [per.tile([128, 512], F32R, name=f"zr{p}", tag=f"zr{p}") for p in range(2)]
        rr = [per.tile([128, 512], F32, name=f"rr{p}", tag=f"rr{p}") for p in range(2)]

        # ---- persistent PSUM (8 banks, region-aliased across phases) ----
        ps = ctx.enter_context(tc.tile_pool(name="ps", bufs=1, space="PSUM"))
        stp = [ps.tile([128, 1024], F32, name=f"stp{p}", tag=f"stp{p}")
               for p in range(2)]
        ua = [ps.tile([128, 512], F32, name=f"ua{p}", tag=f"ua{p}") for p in range(2)]
        ub = [ps.tile([128, 512], F32, name=f"ub{p}", tag=f"ub{p}") for p in range(2)]

        # ---- initial DMAs, ordered so proj(0) can start ASAP ----
        xpool = ctx.enter_context(tc.tile_pool(name="xts", bufs=12))

        def load_x(tb, wq_interleave=False):
            xts = []
            for db in range(8):
                if wq_interleave:
                    nc.sync.dma_start(
                        wq_sb[:, db * 256:(db + 1) * 256],
                        wq_d.ap()[:, db * 256:(db + 1) * 256],
                    )
                xtile = xpool.tile([128, 512], BF16, name="xt", tag="xt")
                nc.sync.dma_start(
                    xtile[:],
                    xt_d.ap()[db * 128:(db + 1) * 128, tb * 512:(tb + 1) * 512],
                )
                xts.append(xtile)
            return xts

        xts_cur = load_x(0, wq_interleave=True)
        nc.sync.dma_start(wk_sb[:], wk_d.ap()[:])
        nc.sync.dma_start(wv_sb[:], wv_d.ap()[:])
        nc.sync.dma_start(bqc_sb[:], bqc_d.ap()[:])
        nc.sync.dma_start(bkc_sb[:], bkc_d.ap()[:])
        nc.sync.dma_start(bvb_sb[:], bvb_d.ap()[:])
        nc.sync.dma_start(e2_sb[:], e2_d.ap()[:])
        nc.sync.dma_start(mtri_sb[:], mtri_d.ap()[:])
        # ones/zero pad columns of the v-tiles (col 64 and 256 of each j blk)
        vt3 = vt[:, 0:NJ * 384].rearrange("p (j c) -> p j c", c=384)
        vsrc = vpad_d.ap()[:].rearrange("p (j c) -> p j c", c=64)
        nc.sync.dma_start(vt3[:, :, 0:64], vsrc)
        nc.sync.dma_start(vt3[:, :, 192:256], vsrc)
        nc.sync.dma_start(vt[:, NJ * 384: NJ * 384 + 64],
                          vpad_d.ap()[:, 0:64])
        nc.sync.dma_start(wo_sb[:], wo_d.ap()[:])

        opool = ctx.enter_context(tc.tile_pool(name="otile", bufs=6))
        epool = ctx.enter_context(tc.tile_pool(name="expt", bufs=14))

        mtri3 = mtri_sb[:].rearrange("p (h w) -> p h w", h=2)

        def proj_mm(out_ps, w_sb, p, xts):
            for db in range(8):
                nc.tensor.matmul(
                    out_ps,
                    w_sb[:, db * 256 + p * 128: db * 256 + (p + 1) * 128],
                    xts[db][:],
                    start=(db == 0), stop=(db == 7),
                )

        def norm_part1(tb1, tail=False):
            # Z rows out of PSUM: Z_a -> zr row 0, Z_b -> zr row 64
            # (at the tail ACT is idle -> use it for the PSUM reads)
            eng = nc.scalar.copy if tail else nc.vector.tensor_copy
            for p in range(2):
                eng(zr[p][0:64, :], ua[p][0:64, :])
                eng(zr[p][64:128, :], ub[p][64:128, :])

        def norm_rb(p, region):
            # e2 matmul: rb rows 0:64 <- Z_b (zr row 64), rows 64:128 <- Z_a
            nc.tensor.matmul(region, e2_sb[:], zr[p][:], start=True, stop=True)

        def norm_part2(p, region):
            nc.vector.reciprocal_approx_fast(out=rr[p][:], in_=region)

        def norm_muls(p, tb1):
            nc.vector.tensor_mul(
                cx[p][0:64, tb1 * 512:(tb1 + 1) * 512],
                ua[p][64:128, :], rr[p][64:128, :],
            )
            nc.vector.tensor_mul(
                cx[p][64:128, tb1 * 512:(tb1 + 1) * 512],
                ub[p][0:64, :], rr[p][0:64, :],
            )

        def oproj_ob(tb1, ob, tail=False):
            slots = [ua[0], ub[0], ua[1], ub[1]] if tail else [ua[0], ub[0]]
            po = slots[ob % len(slots)][:]
            nc.tensor.matmul(
                po,
                wo_sb[:, ob * 128:(ob + 1) * 128],
                cx[0][:, tb1 * 512:(tb1 + 1) * 512],
                start=True, stop=False, skip_group_check=True,
            )
            nc.tensor.matmul(
                po,
                wo_sb[:, 1024 + ob * 128: 1024 + (ob + 1) * 128],
                cx[1][:, tb1 * 512:(tb1 + 1) * 512],
                start=False, stop=True, skip_group_check=True,
            )
            ot_t = opool.tile([128, 512], BF16, name="ot", tag="ot")
            (nc.scalar.copy if tail else nc.vector.tensor_copy)(ot_t[:], po)
            nc.sync.dma_start(
                ot_d.ap()[ob * 128:(ob + 1) * 128, tb1 * 512:(tb1 + 1) * 512],
                ot_t[:],
            )

        def oproj(tb1, tail=False):
            for ob in range(8):
                oproj_ob(tb1, ob, tail=tail)

        for tb in range(NT):
            xts = xts_cur
            if tb + 1 < NT:
                xts_cur = load_x(tb + 1)

            # ---------- projections for tb (+ norm(tb-1) interleaved) ----------
            # psq(p0) -> stp0[:, 0:512]
            if tb > 0:
                norm_part1(tb - 1)
            proj_mm(stp[0][:, 0:512], wq_sb, 0, xts)
            nc.vector.tensor_scalar_add(
                qt[0][:, tb * 512:(tb + 1) * 512], stp[0][:, 0:512],
                bqc_sb[:, 0:1],
            )
            # psq(p1) -> stp0[:, 512:1024]
            proj_mm(stp[0][:, 512:1024], wq_sb, 1, xts)
            nc.vector.tensor_scalar_add(
                qt[1][:, tb * 512:(tb + 1) * 512], stp[0][:, 512:1024],
                bqc_sb[:, 1:2],
            )
            if tb > 0:
                norm_rb(0, stp[0][:, 0:512])
                norm_part2(0, stp[0][:, 0:512])
                norm_muls(0, tb - 1)
            # zero halves of this tb's kt slices (before SDPA(tb) STs)
            for p in range(2):
                nc.vector.memset(kta[p][64:128, tb * 512:(tb + 1) * 512], 0.0)
                nc.vector.memset(ktb[p][0:64, tb * 512:(tb + 1) * 512], 0.0)
            # psk(p0) -> stp1[:, 0:512]
            proj_mm(stp[1][:, 0:512], wk_sb, 0, xts)
            nc.vector.tensor_scalar_add(
                kta[0][0:64, tb * 512:(tb + 1) * 512], stp[1][0:64, 0:512],
                bkc_sb[0:64, 0:1],
            )
            nc.vector.tensor_scalar_add(
                ktb[0][64:128, tb * 512:(tb + 1) * 512], stp[1][64:128, 0:512],
                bkc_sb[64:128, 0:1],
            )
            if tb > 0:
                norm_rb(1, stp[0][:, 512:1024])
                norm_part2(1, stp[0][:, 512:1024])
                norm_muls(1, tb - 1)
            # psk(p1) -> stp1[:, 512:1024]
            proj_mm(stp[1][:, 512:1024], wk_sb, 1, xts)
            nc.vector.tensor_scalar_add(
                kta[1][0:64, tb * 512:(tb + 1) * 512], stp[1][0:64, 512:1024],
                bkc_sb[0:64, 1:2],
            )
            nc.vector.tensor_scalar_add(
                ktb[1][64:128, tb * 512:(tb + 1) * 512], stp[1][64:128, 512:1024],
                bkc_sb[64:128, 1:2],
            )
            # psv: 4 sub-blocks of 128 t-rows -> ua0/ub0/ua1/ub1 [:, 0:256]
            psv_slots = [ua[0], ub[0], ua[1], ub[1]]
            for sub in range(4):
                j = tb * 4 + sub
                psv_t = psv_slots[sub]
                for db in range(8):
                    nc.tensor.matmul(
                        psv_t[:, 0:256],
                        xts[db][:, sub * 128:(sub + 1) * 128],
                        wv_sb[:, db * 256:(db + 1) * 256],
                        start=(db == 0), stop=(db == 7),
                    )
                # scatter v (+bias): v0|v1 -> [64:192], v2|v3 -> [256:384]
                nc.vector.tensor_add(
                    vt[:, j * 384 + 64: j * 384 + 192],
                    psv_t[:, 0:128], bvb_sb[:, 0:128],
                )
                nc.vector.tensor_add(
                    vt[:, j * 384 + 256: j * 384 + 384],
                    psv_t[:, 128:256], bvb_sb[:, 128:256],
                )

            # ---------- SDPA for i = tb; pre-phase STs overlap oproj ----------
            i = tb
            jmax = 4 * i + 4
            ets = {}

            def emit_st_exp(j):
                t = j - 4 * i
                cs = 128 * t if t >= 0 else 0
                w = 512 - cs
                for p in range(2):
                    nc.tensor.matmul(
                        stp[p][:, cs:512],
                        kta[p][:, j * 128:(j + 1) * 128],
                        qt[p][:, i * 512 + cs:(i + 1) * 512],
                        start=True, stop=True,
                    )
                    nc.tensor.matmul(
                        stp[p][:, 512 + cs:1024],
                        ktb[p][:, j * 128:(j + 1) * 128],
                        qt[p][:, i * 512 + cs:(i + 1) * 512],
                        start=True, stop=True,
                    )
                    et = epool.tile([128, 1024], BF16, name="et", tag="et")
                    esrc = stp[p][:].rearrange("p (h w) -> p h w", h=2)[:, :, cs:512]
                    dst = et[:].rearrange("p (h w) -> p h w", h=2)[:, :, cs:512]
                    nc.scalar.activation(dst, esrc, EXP, scale=0.125)
                    if t >= 0:
                        nc.vector.tensor_mul(dst, dst, mtri3[:, :, 0:w])
                    ets[(j, p)] = et

            P = min(5, jmax)
            obq = list(range(8)) if tb > 0 else []
            for j in range(P):
                emit_st_exp(j)
                if j >= 1:
                    for _ in range(2):
                        if obq:
                            oproj_ob(tb - 1, obq.pop(0))
            while obq:
                oproj_ob(tb - 1, obq.pop(0))
            for j in range(P, jmax):
                emit_st_exp(j)
                _pv(nc, vt, ua, ub, ets, j - P, jmax, 4 * i)
                del ets[(j - P, 0)], ets[(j - P, 1)]
            for jt in range(jmax - P, jmax):
                _pv(nc, vt, ua, ub, ets, jt, jmax, 4 * i)

        # tail: norm + oproj for last block
        tb1 = NT - 1
        norm_part1(tb1, tail=True)
        norm_rb(0, stp[0][:, 0:512])
        norm_part2(0, stp[0][:, 0:512])
        norm_muls(0, tb1)
        norm_rb(1, stp[0][:, 512:1024])
        norm_part2(1, stp[0][:, 512:1024])
        norm_muls(1, tb1)
        oproj(tb1, tail=True)

    nc.compile()
    return nc


def _pv(nc, vt, ua, ub, ets, j, jmax, i4):
    t = j - i4
    cs = 128 * t if t >= 0 else 0
    st_flags = dict(start=(j == 0), stop=(j == jmax - 1), skip_group_check=True)
    for p in range(2):
        et = ets[(j, p)]
        base = j * 384 + 192 * p
        nc.tensor.matmul(
            ua[p][:, cs:512], vt[:, base: base + 128], et[:, cs:512], **st_flags
        )
        nc.tensor.matmul(
            ub[p][:, cs:512], vt[:, base + 128: base + 256],
            et[:, 512 + cs:1024], **st_flags
        )


def _get_built(T):
    if T not in _BUILD_CACHE:
        _BUILD_CACHE[T] = _build(T)
    return _BUILD_CACHE[T]


def _rearr_w(w):  # [1024, 256] -> [128, 8*256] (d-block major free dim)
    return np.ascontiguousarray(
        w.reshape(8, 128, 256).transpose(1, 0, 2).reshape(128, 8 * 256)
    )


def _numpy_ref(x, mask, Wq, bq, Wk, bk, Wv, bv, Wo, bo):
    T = x.shape[1]
    q = (x @ Wq + bq).reshape(B, T, H, DK).transpose(0, 2, 1, 3)
    k = (x @ Wk + bk).reshape(B, T, H, DK).transpose(0, 2, 1, 3)
    v = (x @ Wv + bv).reshape(B, T, H, DK).transpose(0, 2, 1, 3)
    s = np.einsum("bhqd,bhkd->bhqk", q, k) / np.sqrt(np.float32(DK))
    s = np.where(mask, s, s - 1e9)
    s = s - s.max(axis=-1, keepdims=True)
    e = np.exp(s)
    p = e / e.sum(axis=-1, keepdims=True)
    o = np.einsum("bhqk,bhkd->bhqd", p, v).transpose(0, 2, 1, 3).reshape(B, T, D)
    return (o @ Wo + bo).astype(np.float32)


def kernel(x, mask, Wq, bq, Wk, bk, Wv, bv, Wo, bo):
    from concourse import bass_utils

    x = np.ascontiguousarray(np.asarray(x, dtype=np.float32))
    mask = np.asarray(mask)
    T = x.shape[1]

    causal = bool(
        np.array_equal(mask[0, 0], np.tril(np.ones((T, T), dtype=bool)))
    )
    if not causal or x.shape != (B, T, D) or T % 512 != 0:
        return _numpy_ref(
            np.asarray(x, np.float32), mask,
            np.asarray(Wq, np.float32), np.asarray(bq, np.float32),
            np.asarray(Wk, np.float32), np.asarray(bk, np.float32),
            np.asarray(Wv, np.float32), np.asarray(bv, np.float32),
            np.asarray(Wo, np.float32), np.asarray(bo, np.float32),
        )

    in_maps = _make_in_maps(dict(x=x, Wq=Wq, bq=bq, Wk=Wk, bk=bk,
                                 Wv=Wv, bv=bv, Wo=Wo))
    nc = _get_built(T)
    res = bass_utils.run_bass_kernel_spmd(nc, in_maps, core_ids=list(range(NCORES)))

    out = np.zeros((B, T, D), np.float32)
    for c in range(NCORES):
        out[c // 4] += res.results[c]["ot"].T.astype(np.float32)
    out += np.asarray(bo, np.float32)
    return out


def _make_in_maps(inputs):
    import ml_dtypes

    x = np.ascontiguousarray(np.asarray(inputs["x"], np.float32))
    T = x.shape[1]
    NJ = T // 128
    Wq = np.asarray(inputs["Wq"], np.float32)
    Wk = np.asarray(inputs["Wk"], np.float32)
    Wv = np.asarray(inputs["Wv"], np.float32)
    Wo = np.asarray(inputs["Wo"], np.float32)
    bq = np.asarray(inputs["bq"], np.float32)
    bk = np.asarray(inputs["bk"], np.float32)
    bv = np.asarray(inputs["bv"], np.float32)

    e2 = np.zeros((128, 128), np.float32)
    e2[64, 0:64] = 1.0
    e2[0, 64:128] = 1.0
    vpad = np.zeros((128, NJ * 64), ml_dtypes.bfloat16)
    vpad[:, ::64] = 1.0
    mtri = np.zeros((128, 1024), ml_dtypes.bfloat16)
    tri = (np.arange(128)[:, None] <= np.arange(512)[None, :])
    mtri[:, 0:512] = tri
    mtri[:, 512:1024] = tri

    xts = [np.ascontiguousarray(x[b].T.astype(ml_dtypes.bfloat16))
           for b in range(B)]

    in_maps = []
    for c in range(NCORES):
        b, g = divmod(c, 4)
        cols = slice(g * CW, (g + 1) * CW)
        rows = slice(g * CW, (g + 1) * CW)
        wo_g = Wo[rows]  # [256, 1024]
        in_maps.append({
            "xt": xts[b],
            "wq": _rearr_w(Wq[:, cols]).astype(ml_dtypes.bfloat16),
            "wk": _rearr_w(Wk[:, cols]).astype(ml_dtypes.bfloat16),
            "wv": _rearr_w(Wv[:, cols]).astype(ml_dtypes.bfloat16),
            "wo": np.ascontiguousarray(
                wo_g.reshape(2, 128, 1024).transpose(1, 0, 2).reshape(128, 2048)
            ).astype(ml_dtypes.bfloat16),
            "bqc": np.ascontiguousarray(bq[cols].reshape(2, 128).T),
            "bkc": np.ascontiguousarray(bk[cols].reshape(2, 128).T),
            "bvb": np.ascontiguousarray(
                np.broadcast_to(bv[cols][None, :], (128, 256)).copy()
            ),
            "e2sel": e2,
            "vpad": vpad,
            "mtri": mtri,
        })

    return in_maps


# revision 9
# speedup vs baseline: 1.0776x; 1.0027x over previous
"""Multi-head causal attention (B=2, T=4096, D=1024, H=16) on 8 trn2 cores.

Sharding: core c = 4*b + g handles batch b and head-group g (4 heads).
Merged single-pass pipeline per 512-col q-block tb:
  proj(tb) -> norm(tb-1) -> oproj(tb-1) -> SDPA(i=tb)
SDPA streams in bf16 (q/k/v/probs); Z rows fused into PV via [v|ones]
stationary tiles; causal mask via DVE multiply with a triangular bf16
constant. Host sums the per-core partial O^T and adds bo.
"""
import numpy as np

B, T0, D, H = 2, 4096, 1024, 16
DK = D // H          # 64
NCORES = 8
HPC = H // 4         # 4 heads per core
CW = HPC * DK        # 256 head-columns per core

_BUILD_CACHE = {}


def _build(T):
    import concourse.bacc as bacc
    import concourse.mybir as mybir
    import concourse.tile as tile
    from contextlib import ExitStack

    F32 = mybir.dt.float32
    F32R = mybir.dt.float32r
    BF16 = mybir.dt.bfloat16
    EXP = mybir.ActivationFunctionType.Exp

    NT = T // 512    # q-blocks of 512
    NJ = T // 128    # k-blocks of 128
    VTW = NJ * 384 + 64   # per j: ones|v0|v1|ones2|v2|v3, plus final ones

    nc = bacc.Bacc("TRN2", target_bir_lowering=False, debug=False, num_devices=8)

    xt_d = nc.dram_tensor("xt", [D, T], BF16, kind="ExternalInput")
    wq_d = nc.dram_tensor("wq", [128, 8 * 256], BF16, kind="ExternalInput")
    wk_d = nc.dram_tensor("wk", [128, 8 * 256], BF16, kind="ExternalInput")
    wv_d = nc.dram_tensor("wv", [128, 8 * 256], BF16, kind="ExternalInput")
    wo_d = nc.dram_tensor("wo", [128, 2 * 1024], BF16, kind="ExternalInput")
    bqc_d = nc.dram_tensor("bqc", [128, 2], F32, kind="ExternalInput")
    bkc_d = nc.dram_tensor("bkc", [128, 2], F32, kind="ExternalInput")
    bvb_d = nc.dram_tensor("bvb", [128, 256], F32, kind="ExternalInput")
    e2_d = nc.dram_tensor("e2sel", [128, 128], F32R, kind="ExternalInput")
    vpad_d = nc.dram_tensor("vpad", [128, NJ * 64], BF16, kind="ExternalInput")
    mtri_d = nc.dram_tensor("mtri", [128, 1024], BF16, kind="ExternalInput")
    ot_d = nc.dram_tensor("ot", [D, T], BF16, kind="ExternalOutput")

    with tile.TileContext(nc) as tc, ExitStack() as ctx:
        ctx.enter_context(nc.allow_low_precision(reason="bf16/fp32r by design"))

        # ---- persistent SBUF ----
        per = ctx.enter_context(tc.tile_pool(name="persist", bufs=1))
        qt = [per.tile([128, T], BF16, name=f"qt{p}", tag=f"qt{p}") for p in range(2)]
        kta = [per.tile([128, T], BF16, name=f"kta{p}", tag=f"kta{p}") for p in range(2)]
        ktb = [per.tile([128, T], BF16, name=f"ktb{p}", tag=f"ktb{p}") for p in range(2)]
        vt = per.tile([128, VTW], BF16, name="vt", tag="vt")
        wq_sb = per.tile([128, 2048], BF16, name="wq", tag="wq")
        wk_sb = per.tile([128, 2048], BF16, name="wk", tag="wk")
        wv_sb = per.tile([128, 2048], BF16, name="wv", tag="wv")
        wo_sb = per.tile([128, 2048], BF16, name="wo", tag="wo")
        e2_sb = per.tile([128, 128], F32R, name="e2", tag="e2")
        mtri_sb = per.tile([128, 1024], BF16, name="mtri", tag="mtri")
        bqc_sb = per.tile([128, 2], F32, name="bqc", tag="bqc")
        bkc_sb = per.tile([128, 2], F32, name="bkc", tag="bkc")
        bvb_sb = per.tile([128, 256], F32, name="bvb", tag="bvb")
        cx = [per.tile([128, T], BF16, name=f"cx{p}", tag=f"cx{p}") for p in range(2)]
        zr = [per.tile([128, 512], F32R, name=f"zr{p}", tag=f"zr{p}") for p in range(2)]
        rr = [per.tile([128, 512], F32, name=f"rr{p}", tag=f"rr{p}") for p in range(2)]

        # ---- persistent PSUM (8 banks, region-aliased across phases) ----
        ps = ctx.enter_context(tc.tile_pool(name="ps", bufs=1, space="PSUM"))
        stp = [ps.tile([128, 1024], F32, name=f"stp{p}", tag=f"stp{p}")
               for p in range(2)]
        ua = [ps.tile([128, 512], F32, name=f"ua{p}", tag=f"ua{p}") for p in range(2)]
        ub = [ps.tile([128, 512], F32, name=f"ub{p}", tag=f"ub{p}") for p in range(2)]

        # ---- initial DMAs, ordered so proj(0) can start ASAP ----
        xpool = ctx.enter_context(tc.tile_pool(name="xts", bufs=12))

        def load_x(tb, wq_interleave=False):
            xts = []
            for db in range(8):
                if wq_interleave:
                    nc.sync.dma_start(
                        wq_sb[:, db * 256:(db + 1) * 256],
                        wq_d.ap()[:, db * 256:(db + 1) * 256],
                    )
                xtile = xpool.tile([128, 512], BF16, name="xt", tag="xt")
                nc.sync.dma_start(
                    xtile[:],
                    xt_d.ap()[db * 128:(db + 1) * 128, tb * 512:(tb + 1) * 512],
                )
                xts.append(xtile)
            return xts

        xts_cur = load_x(0, wq_interleave=True)
        nc.sync.dma_start(wk_sb[:], wk_d.ap()[:])
        nc.sync.dma_start(wv_sb[:], wv_d.ap()[:])
        nc.sync.dma_start(bqc_sb[:], bqc_d.ap()[:])
        nc.sync.dma_start(bkc_sb[:], bkc_d.ap()[:])
        nc.sync.dma_start(bvb_sb[:], bvb_d.ap()[:])
        nc.sync.dma_start(e2_sb[:], e2_d.ap()[:])
        nc.sync.dma_start(mtri_sb[:], mtri_d.ap()[:])
        # ones/zero pad columns of the v-tiles (col 64 and 256 of each j blk)
        vt3 = vt[:, 0:NJ * 384].rearrange("p (j c) -> p j c", c=384)
        vsrc = vpad_d.ap()[:].rearrange("p (j c) -> p j c", c=64)
        nc.sync.dma_start(vt3[:, :, 0:64], vsrc)
        nc.sync.dma_start(vt3[:, :, 192:256], vsrc)
        nc.sync.dma_start(vt[:, NJ * 384: NJ * 384 + 64],
                          vpad_d.ap()[:, 0:64])
        nc.sync.dma_start(wo_sb[:], wo_d.ap()[:])

        opool = ctx.enter_context(tc.tile_pool(name="otile", bufs=6))
        epool = ctx.enter_context(tc.tile_pool(name="expt", bufs=14))

        mtri3 = mtri_sb[:].rearrange("p (h w) -> p h w", h=2)

        def proj_mm(out_ps, w_sb, p, xts):
            for db in range(8):
                nc.tensor.matmul(
                    out_ps,
                    w_sb[:, db * 256 + p * 128: db * 256 + (p + 1) * 128],
                    xts[db][:],
                    start=(db == 0), stop=(db == 7),
                )

        def norm_part1(tb1, tail=False):
            # Z rows out of PSUM: Z_a -> zr row 0, Z_b -> zr row 64
            # (at the tail ACT is idle -> use it for the PSUM reads)
            eng = nc.scalar.copy if tail else nc.vector.tensor_copy
            for p in range(2):
                eng(zr[p][0:64, :], ua[p][0:64, :])
                eng(zr[p][64:128, :], ub[p][64:128, :])

        def norm_rb(p, region):
            # e2 matmul: rb rows 0:64 <- Z_b (zr row 64), rows 64:128 <- Z_a
            nc.tensor.matmul(region, e2_sb[:], zr[p][:], start=True, stop=True)

        def norm_part2(p, region):
            nc.vector.reciprocal_approx_fast(out=rr[p][:], in_=region)

        def norm_muls(p, tb1):
            nc.vector.tensor_mul(
                cx[p][0:64, tb1 * 512:(tb1 + 1) * 512],
                ua[p][64:128, :], rr[p][64:128, :],
            )
            nc.vector.tensor_mul(
                cx[p][64:128, tb1 * 512:(tb1 + 1) * 512],
                ub[p][0:64, :], rr[p][0:64, :],
            )

        def oproj_ob(tb1, ob, tail=False):
            slots = [ua[0], ub[0], ua[1], ub[1]] if tail else [ua[0], ub[0]]
            po = slots[ob % len(slots)][:]
            nc.tensor.matmul(
                po,
                wo_sb[:, ob * 128:(ob + 1) * 128],
                cx[0][:, tb1 * 512:(tb1 + 1) * 512],
                start=True, stop=False, skip_group_check=True,
            )
            nc.tensor.matmul(
                po,
                wo_sb[:, 1024 + ob * 128: 1024 + (ob + 1) * 128],
                cx[1][:, tb1 * 512:(tb1 + 1) * 512],
                start=False, stop=True, skip_group_check=True,
            )
            ot_t = opool.tile([128, 512], BF16, name="ot", tag="ot")
            (nc.scalar.copy if tail else nc.vector.tensor_copy)(ot_t[:], po)
            nc.sync.dma_start(
                ot_d.ap()[ob * 128:(ob + 1) * 128, tb1 * 512:(tb1 + 1) * 512],
                ot_t[:],
            )

        def oproj(tb1, tail=False):
            for ob in range(8):
                oproj_ob(tb1, ob, tail=tail)

        for tb in range(NT):
            xts = xts_cur
            if tb + 1 < NT:
                xts_cur = load_x(tb + 1)

            # ---------- projections for tb (+ norm(tb-1) interleaved) ----------
            # psq(p0) -> stp0[:, 0:512]
            if tb > 0:
                norm_part1(tb - 1)
            proj_mm(stp[0][:, 0:512], wq_sb, 0, xts)
            nc.vector.tensor_scalar_add(
                qt[0][:, tb * 512:(tb + 1) * 512], stp[0][:, 0:512],
                bqc_sb[:, 0:1],
            )
            # psq(p1) -> stp0[:, 512:1024]
            proj_mm(stp[0][:, 512:1024], wq_sb, 1, xts)
            nc.vector.tensor_scalar_add(
                qt[1][:, tb * 512:(tb + 1) * 512], stp[0][:, 512:1024],
                bqc_sb[:, 1:2],
            )
            if tb > 0:
                norm_rb(0, stp[0][:, 0:512])
                norm_part2(0, stp[0][:, 0:512])
                norm_muls(0, tb - 1)
            # zero halves of this tb's kt slices (before SDPA(tb) STs)
            for p in range(2):
                nc.gpsimd.memset(kta[p][64:128, tb * 512:(tb + 1) * 512], 0.0)
                nc.gpsimd.memset(ktb[p][0:64, tb * 512:(tb + 1) * 512], 0.0)
            # psk(p0) -> stp1[:, 0:512]
            proj_mm(stp[1][:, 0:512], wk_sb, 0, xts)
            nc.vector.tensor_scalar_add(
                kta[0][0:64, tb * 512:(tb + 1) * 512], stp[1][0:64, 0:512],
                bkc_sb[0:64, 0:1],
            )
            nc.vector.tensor_scalar_add(
                ktb[0][64:128, tb * 512:(tb + 1) * 512], stp[1][64:128, 0:512],
                bkc_sb[64:128, 0:1],
            )
            if tb > 0:
                norm_rb(1, stp[0][:, 512:1024])
                norm_part2(1, stp[0][:, 512:1024])
                norm_muls(1, tb - 1)
            # psk(p1) -> stp1[:, 512:1024]
            proj_mm(stp[1][:, 512:1024], wk_sb, 1, xts)
            nc.vector.tensor_scalar_add(
                kta[1][0:64, tb * 512:(tb + 1) * 512], stp[1][0:64, 512:1024],
                bkc_sb[0:64, 1:2],
            )
            nc.vector.tensor_scalar_add(
                ktb[1][64:128, tb * 512:(tb + 1) * 512], stp[1][64:128, 512:1024],
                bkc_sb[64:128, 1:2],
            )
            # psv: 4 sub-blocks of 128 t-rows -> ua0/ub0/ua1/ub1 [:, 0:256]
            psv_slots = [ua[0], ub[0], ua[1], ub[1]]
            for sub in range(4):
                j = tb * 4 + sub
                psv_t = psv_slots[sub]
                for db in range(8):
                    nc.tensor.matmul(
                        psv_t[:, 0:256],
                        xts[db][:, sub * 128:(sub + 1) * 128],
                        wv_sb[:, db * 256:(db + 1) * 256],
                        start=(db == 0), stop=(db == 7),
                    )
                # scatter v (+bias): v0|v1 -> [64:192], v2|v3 -> [256:384]
                nc.vector.tensor_add(
                    vt[:, j * 384 + 64: j * 384 + 192],
                    psv_t[:, 0:128], bvb_sb[:, 0:128],
                )
                nc.vector.tensor_add(
                    vt[:, j * 384 + 256: j * 384 + 384],
                    psv_t[:, 128:256], bvb_sb[:, 128:256],
                )

            # ---------- SDPA for i = tb; pre-phase STs overlap oproj ----------
            i = tb
            jmax = 4 * i + 4
            ets = {}

            def emit_st_exp(j):
                t = j - 4 * i
                cs = 128 * t if t >= 0 else 0
                w = 512 - cs
                for p in range(2):
                    nc.tensor.matmul(
                        stp[p][:, cs:512],
                        kta[p][:, j * 128:(j + 1) * 128],
                        qt[p][:, i * 512 + cs:(i + 1) * 512],
                        start=True, stop=True,
                    )
                    nc.tensor.matmul(
                        stp[p][:, 512 + cs:1024],
                        ktb[p][:, j * 128:(j + 1) * 128],
                        qt[p][:, i * 512 + cs:(i + 1) * 512],
                        start=True, stop=True,
                    )
                    et = epool.tile([128, 1024], BF16, name="et", tag="et")
                    esrc = stp[p][:].rearrange("p (h w) -> p h w", h=2)[:, :, cs:512]
                    dst = et[:].rearrange("p (h w) -> p h w", h=2)[:, :, cs:512]
                    nc.scalar.activation(dst, esrc, EXP, scale=0.125)
                    if t >= 0:
                        nc.vector.tensor_mul(dst, dst, mtri3[:, :, 0:w])
                    ets[(j, p)] = et

            P = min(5, jmax)
            obq = list(range(8)) if tb > 0 else []
            for j in range(P):
                emit_st_exp(j)
                if j >= 1:
                    for _ in range(2):
                        if obq:
                            oproj_ob(tb - 1, obq.pop(0))
            while obq:
                oproj_ob(tb - 1, obq.pop(0))
            for j in range(P, jmax):
                emit_st_exp(j)
                _pv(nc, vt, ua, ub, ets, j - P, jmax, 4 * i)
                del ets[(j - P, 0)], ets[(j - P, 1)]
            for jt in range(jmax - P, jmax):
                _pv(nc, vt, ua, ub, ets, jt, jmax, 4 * i)

        # tail: norm + oproj for last block
        tb1 = NT - 1
        norm_part1(tb1, tail=True)
        norm_rb(0, stp[0][:, 0:512])
        norm_part2(0, stp[0][:, 0:512])
        norm_muls(0, tb1)
        norm_rb(1, stp[0][:, 512:1024])
        norm_part2(1, stp[0][:, 512:1024])
        norm_muls(1, tb1)
        oproj(tb1, tail=True)

    nc.compile()
    return nc


def _pv(nc, vt, ua, ub, ets, j, jmax, i4):
    t = j - i4
    cs = 128 * t if t >= 0 else 0
    st_flags = dict(start=(j == 0), stop=(j == jmax - 1), skip_group_check=True)
    for p in range(2):
        et = ets[(j, p)]
        base = j * 384 + 192 * p
        nc.tensor.matmul(
            ua[p][:, cs:512], vt[:, base: base + 128], et[:, cs:512], **st_flags
        )
        nc.tensor.matmul(
            ub[p][:, cs:512], vt[:, base + 128: base + 256],
            et[:, 512 + cs:1024], **st_flags
        )


def _get_built(T):
    if T not in _BUILD_CACHE:
        _BUILD_CACHE[T] = _build(T)
    return _BUILD_CACHE[T]


def _rearr_w(w):  # [1024, 256] -> [128, 8*256] (d-block major free dim)
    return np.ascontiguousarray(
        w.reshape(8, 128, 256).transpose(1, 0, 2).reshape(128, 8 * 256)
    )


def _numpy_ref(x, mask, Wq, bq, Wk, bk, Wv, bv, Wo, bo):
    T = x.shape[1]
    q = (x @ Wq + bq).reshape(B, T, H, DK).transpose(0, 2, 1, 3)
    k = (x @ Wk + bk).reshape(B, T, H, DK).transpose(0, 2, 1, 3)
    v = (x @ Wv + bv).reshape(B, T, H, DK).transpose(0, 2, 1, 3)
    s = np.einsum("bhqd,bhkd->bhqk", q, k) / np.sqrt(np.float32(DK))
    s = np.where(mask, s, s - 1e9)
    s = s - s.max(axis=-1, keepdims=True)
    e = np.exp(s)
    p = e / e.sum(axis=-1, keepdims=True)
    o = np.einsum("bhqk,bhkd->bhqd", p, v).transpose(0, 2, 1, 3).reshape(B, T, D)
    return (o @ Wo + bo).astype(np.float32)


def kernel(x, mask, Wq, bq, Wk, bk, Wv, bv, Wo, bo):
    from concourse import bass_utils

    x = np.ascontiguousarray(np.asarray(x, dtype=np.float32))
    mask = np.asarray(mask)
    T = x.shape[1]

    causal = bool(
        np.array_equal(mask[0, 0], np.tril(np.ones((T, T), dtype=bool)))
    )
    if not causal or x.shape != (B, T, D) or T % 512 != 0:
        return _numpy_ref(
            np.asarray(x, np.float32), mask,
            np.asarray(Wq, np.float32), np.asarray(bq, np.float32),
            np.asarray(Wk, np.float32), np.asarray(bk, np.float32),
            np.asarray(Wv, np.float32), np.asarray(bv, np.float32),
            np.asarray(Wo, np.float32), np.asarray(bo, np.float32),
        )

    in_maps = _make_in_maps(dict(x=x, Wq=Wq, bq=bq, Wk=Wk, bk=bk,
                                 Wv=Wv, bv=bv, Wo=Wo))
    nc = _get_built(T)
    res = bass_utils.run_bass_kernel_spmd(nc, in_maps, core_ids=list(range(NCORES)))

    out = np.zeros((B, T, D), np.float32)
    for c in range(NCORES):
        out[c // 4] += res.results[c]["ot"].T.astype(np.float32)
    out += np.asarray(bo, np.float32)
    return out


def _make_in_maps(inputs):
    import ml_dtypes

    x = np.ascontiguousarray(np.asarray(inputs["x"], np.float32))
    T = x.shape[1]
    NJ = T // 128
    Wq = np.asarray(inputs["Wq"], np.float32)
    Wk = np.asarray(inputs["Wk"], np.float32)
    Wv = np.asarray(inputs["Wv"], np.float32)
    Wo = np.asarray(inputs["Wo"], np.float32)
    bq = np.asarray(inputs["bq"], np.float32)
    bk = np.asarray(inputs["bk"], np.float32)
    bv = np.asarray(inputs["bv"], np.float32)

    e2 = np.zeros((128, 128), np.float32)
    e2[64, 0:64] = 1.0
    e2[0, 64:128] = 1.0
    vpad = np.zeros((128, NJ * 64), ml_dtypes.bfloat16)
    vpad[:, ::64] = 1.0
    mtri = np.zeros((128, 1024), ml_dtypes.bfloat16)
    tri = (np.arange(128)[:, None] <= np.arange(512)[None, :])
    mtri[:, 0:512] = tri
    mtri[:, 512:1024] = tri

    xts = [np.ascontiguousarray(x[b].T.astype(ml_dtypes.bfloat16))
           for b in range(B)]

    in_maps = []
    for c in range(NCORES):
        b, g = divmod(c, 4)
        cols = slice(g * CW, (g + 1) * CW)
        rows = slice(g * CW, (g + 1) * CW)
        wo_g = Wo[rows]  # [256, 1024]
        in_maps.append({
            "xt": xts[b],
            "wq": _rearr_w(Wq[:, cols]).astype(ml_dtypes.bfloat16),
            "wk": _rearr_w(Wk[:, cols]).astype(ml_dtypes.bfloat16),
            "wv": _rearr_w(Wv[:, cols]).astype(ml_dtypes.bfloat16),
            "wo": np.ascontiguousarray(
                wo_g.reshape(2, 128, 1024).transpose(1, 0, 2).reshape(128, 2048)
            ).astype(ml_dtypes.bfloat16),
            "bqc": np.ascontiguousarray(bq[cols].reshape(2, 128).T),
            "bkc": np.ascontiguousarray(bk[cols].reshape(2, 128).T),
            "bvb": np.ascontiguousarray(
                np.broadcast_to(bv[cols][None, :], (128, 256)).copy()
            ),
            "e2sel": e2,
            "vpad": vpad,
            "mtri": mtri,
        })

    return in_maps
